# revision 15
# baseline (speedup 1.0000x reference)
"""Trainium2 Bass kernel: multi-head attention with sequence-axis layernorm
and relative position bias, sharded 8-way over heads (2 heads/core).

v2 layout strategy (per core):
  - LN over sequence axis in [d_partition, n_free] layout; stats on DVE
    (bn_stats/bn_aggr), apply on DVE via fused tensor_scalar (x*scl + nshf)
    in bf16 (4x mode); g folded into Wq/Wkv on the host.
  - qT/kT [inner_local=128, b*n] via const-weight matmuls (K=128, Nf=512).
  - v natural per (b, nj): va_full[b,nj] [128 tokens, 128 inner] bf16.
  - attention rounds (ni, nj): all 4 streams (b x h) share ONE persistent
    4-bank PSUM tile [128, 2048] f32, cols [b0h0|b0h1|b1h0|b1h1]; the two
    sims of a batch are row-tiled (K=64 at row groups 0/64) into DIFFERENT
    banks and run concurrently on the PE.
  - exp: ONE ScalarE activation per batch-pair [128, 1024] spanning 2 PSUM
    banks (amortizes the ~352-cycle ACT instruction overhead).
  - bias folded multiplicatively: host precomputes exp(biasT) bf16; DVE
    multiplies (2x mode) into au.
  - AV: col-tiled pairs — h0 -> pav[b][0:64], h1 -> pav[b][64:128] (M=64,
    col groups disjoint -> concurrent), accumulated over nj. This yields a
    head-STACKED av [128, qi] enabling a K=128 output projection.
  - Z: separate [128,512] PSUM bank; 4 concurrent M=1 col-tiled matmuls
    with ones-weights at partitions {0,32,64,96}, accumulated over nj.
  - softmax denominator: reciprocal on DVE at ni boundary, DRAM roundtrip
    to broadcast 1/Z rows across 64 partitions; normalization deferred to
    the out-projection phase (off the attention critical path).
  - out-proj: stacked K=128 matmuls (lhsT = av_n [128, tok]), PSUM->SBUF
    copies alternating DVE/ACT, bf16 partial output summed on host in f32.
"""

import numpy as np
import ml_dtypes

import concourse.bass as bass
from concourse import bacc
import concourse.mybir as mybir
import concourse.tile as tile
from concourse.bass_utils import run_bass_kernel_spmd

F32 = mybir.dt.float32
BF16 = mybir.dt.bfloat16
BF = ml_dtypes.bfloat16
AF = mybir.ActivationFunctionType
ALU = mybir.AluOpType

# full-size problem constants
B, N, DIM = 2, 2048, 1024
HEADS, DH = 16, 64
NCORES = 8
HL = HEADS // NCORES          # heads per core = 2
IL = HL * DH                  # local inner = 128
INNER = HEADS * DH            # 1024


def build(b_sz=B, n_sz=N, dim=DIM, eps=1e-5):
    """Build the per-core Bass graph (SPMD across 8 cores)."""
    nd = dim // 128               # d tiles
    nch = (b_sz * n_sz) // 512    # 512-col chunks of flattened b*n
    njb = n_sz // 128             # key tiles per batch
    nic = n_sz // 512             # query chunks per batch
    bn = b_sz * n_sz
    nsub = n_sz // 512            # bn_stats subgroups

    nc = bacc.Bacc(None, target_bir_lowering=False)
    xT = nc.declare_dram_parameter("xT", [b_sz, dim, n_sz], BF16, isOutput=False)
    wqT = nc.declare_dram_parameter("wqT", [dim, IL], BF16, isOutput=False)
    wkT = nc.declare_dram_parameter("wkT", [dim, IL], BF16, isOutput=False)
    wvT = nc.declare_dram_parameter("wvT", [dim, IL], BF16, isOutput=False)
    woT = nc.declare_dram_parameter("woT", [IL, dim], BF16, isOutput=False)
    biasT = nc.declare_dram_parameter("biasT", [HL, n_sz, n_sz], BF16, isOutput=False)  # exp(bias.T)
    out = nc.declare_dram_parameter("out", [bn, dim], BF16, isOutput=True)
    zdram = nc.dram_tensor("zscratch", [b_sz, HL, 1, n_sz], BF16)

    with tile.TileContext(nc) as tc:
        with (
            tc.tile_pool(name="consts", bufs=1) as consts,
            tc.tile_pool(name="persist", bufs=1) as persist,
        ):
            # ---- load weights ----
            wq_s, wk_s, wv_s = [], [], []
            for dt in range(nd):
                for lst, src, nm in ((wq_s, wqT, "wq"), (wk_s, wkT, "wk"), (wv_s, wvT, "wv")):
                    t = consts.tile([128, IL], BF16, tag=f"{nm}{dt}")
                    nc.sync.dma_start(out=t, in_=src[dt * 128:(dt + 1) * 128, :])
                    lst.append(t)
            wo_full = consts.tile([IL, dim], BF16, tag="wo")
            nc.sync.dma_start(out=wo_full, in_=woT[:, :])
            ones = consts.tile([128, 1], BF16, tag="ones")
            nc.vector.memset(ones, 1.0)

            xn = {}
            qT = persist.tile([IL, bn], BF16, tag="qT")
            kT = persist.tile([IL, bn], BF16, tag="kT")
            va = {}   # (b, nj) -> [128 tokens, 128 inner] bf16

            # ---- Phase 1: layernorm over sequence axis ----
            # mean/var via DVE bn_stats for half the tiles, via ScalarE
            # Square/Identity+accum_out for the other half (the two engines
            # run the stats concurrently; DVE was the pre-phase bottleneck)
            inv_n = 1.0 / n_sz
            with (
                tc.tile_pool(name="xload", bufs=4) as xload,
                tc.tile_pool(name="lns", bufs=8) as lns,
                tc.tile_pool(name="lnscr", bufs=2) as lnscr,
            ):
                for b in range(b_sz):
                    for dt in range(nd):
                        xt = xload.tile([128, n_sz], BF16, tag="xt",
                                        name=f"xt_{b}_{dt}")
                        nc.sync.dma_start(out=xt, in_=xT[b, dt * 128:(dt + 1) * 128, :])
                        mv = lns.tile([128, 2], F32, tag="mv", name=f"mv_{b}_{dt}")
                        if (b * nd + dt) % 2 == 0:
                            stats = lns.tile([128, nsub, 6], F32, tag="stats",
                                             name=f"st_{b}_{dt}")
                            for s in range(nsub):
                                nc.vector.bn_stats(out=stats[:, s, :],
                                                   in_=xt[:, s * 512:(s + 1) * 512])
                            nc.vector.bn_aggr(out=mv, in_=stats)
                        else:
                            scr = lnscr.tile([128, n_sz], BF16, tag="scr",
                                             name=f"scr_{b}_{dt}")
                            sums = lns.tile([128, 2], F32, tag="sums",
                                            name=f"sums_{b}_{dt}")
                            nc.scalar.activation(out=scr, in_=xt, func=AF.Identity,
                                                 accum_out=sums[:, 0:1])
                            nc.scalar.activation(out=scr, in_=xt, func=AF.Square,
                                                 accum_out=sums[:, 1:2])
                            # mean = sum/n ; var = sumsq/n - mean^2
                            nc.vector.tensor_scalar_mul(mv[:, 0:1], sums[:, 0:1], inv_n)
                            msq = lns.tile([128, 1], F32, tag="msq",
                                           name=f"msq_{b}_{dt}")
                            nc.vector.tensor_mul(msq, mv[:, 0:1], mv[:, 0:1])
                            with nc.allow_low_precision(reason="var f32"):
                                nc.vector.tensor_scalar(
                                    mv[:, 1:2], sums[:, 1:2], inv_n, msq,
                                    ALU.mult, ALU.subtract)
                        vmax = lns.tile([128, 1], F32, tag="vmax", name=f"vm_{b}_{dt}")
                        nc.vector.tensor_scalar_max(vmax, mv[:, 1:2], eps)
                        sq = lns.tile([128, 1], F32, tag="sq", name=f"sq_{b}_{dt}")
                        nc.scalar.activation(out=sq, in_=vmax, func=AF.Sqrt)
                        scl = lns.tile([128, 1], F32, tag="scl", name=f"scl_{b}_{dt}")
                        nc.vector.reciprocal(scl, sq)
                        nshf = lns.tile([128, 1], F32, tag="nshf", name=f"ns_{b}_{dt}")
                        with nc.allow_low_precision(reason="mean*scl in f32; fine"):
                            nc.vector.tensor_scalar(
                                nshf, mv[:, 0:1], scl, -1.0, ALU.mult, ALU.mult)
                        xnt = persist.tile([128, n_sz], BF16, tag=f"xn_{b}_{dt}")
                        with nc.allow_low_precision(reason="bf16 LN apply; ~4e-3 ok"):
                            nc.vector.tensor_scalar(
                                xnt, xt, scl, nshf, ALU.mult, ALU.add)
                        xn[b, dt] = xnt

            # ---- Phase 2a: q/k projections (transposed layout) ----
            with tc.tile_pool(name="pproj", bufs=4, space="PSUM") as pproj:
                for ch in range(nch):
                    b = (ch * 512) // n_sz
                    col0 = (ch * 512) % n_sz
                    for (w_s, dst) in ((wq_s, qT), (wk_s, kT)):
                        ps = pproj.tile([IL, 512], F32, tag="ps")
                        for dt in range(nd):
                            nc.tensor.matmul(
                                ps, w_s[dt], xn[b, dt][:, col0:col0 + 512],
                                start=(dt == 0), stop=(dt == nd - 1),
                            )
                        nc.scalar.activation(out=dst[:, ch * 512:(ch + 1) * 512],
                                             in_=ps, func=AF.Copy)

            # ---- Phase 2b: v via vT (const weights, Nf=512) + DMA transpose ----
            vTs = persist.tile([IL, bn], BF16, tag="vTs")
            with tc.tile_pool(name="pv", bufs=4, space="PSUM") as pv:
                for ch in range(nch):
                    b = (ch * 512) // n_sz
                    col0 = (ch * 512) % n_sz
                    psv = pv.tile([IL, 512], F32, tag="psv", name=f"psv_{ch}")
                    for dt in range(nd):
                        nc.tensor.matmul(
                            psv, wv_s[dt], xn[b, dt][:, col0:col0 + 512],
                            start=(dt == 0), stop=(dt == nd - 1),
                        )
                    nc.scalar.activation(out=vTs[:, ch * 512:(ch + 1) * 512],
                                         in_=psv, func=AF.Copy)
                for b in range(b_sz):
                    for nj in range(njb):
                        t = persist.tile([128, IL], BF16, tag=f"va_{b}_{nj}")
                        nc.sync.dma_start_transpose(
                            out=t,
                            in_=vTs[:, b * n_sz + nj * 128:b * n_sz + (nj + 1) * 128])
                        va[b, nj] = t

            # ---- Phase 3: attention ----
            # stream -> psim column range: [b0h0 | b0h1 | b1h0 | b1h1]
            # (each 512 f32 = exactly one PSUM bank; b-pairs adjacent so one
            # 1024-wide exp covers both heads of a batch)
            av_u = {b: persist.tile([128, n_sz], BF16, tag=f"avu_{b}",
                                    name=f"avu_{b}")
                    for b in range(b_sz)}
            av_n = {b: persist.tile([128, n_sz], BF16, tag=f"avn_{b}",
                                    name=f"avn_{b}")
                    for b in range(b_sz)}
            zbb = {}  # (b, ni) -> [128, 512] bf16 stacked 1/Z broadcast
            with (
                tc.tile_pool(name="psim", bufs=1, space="PSUM") as psimp,
                tc.tile_pool(name="pav", bufs=1, space="PSUM") as pavp,
                tc.tile_pool(name="pz", bufs=1, space="PSUM") as pzp,
                tc.tile_pool(name="pop", bufs=1, space="PSUM") as popp,
                tc.tile_pool(name="biasp", bufs=4) as biasp,
                tc.tile_pool(name="aep", bufs=1) as aep,
                tc.tile_pool(name="osp", bufs=4) as osp,
                tc.tile_pool(name="zc", bufs=4) as zc,
            ):
                # two 2-bank sim buffers: exp(r) reads one while the next
                # round's sims fill the other (the exp pipeline never stalls)
                psim = [psimp.tile([128, HL * 512], F32, tag=f"psim{i}",
                                   name=f"psim{i}") for i in range(2)]
                ae = [aep.tile([128, HL * 512], BF16, tag=f"ae{i}", name=f"ae{i}")
                      for i in range(3)]
                au = [aep.tile([128, HL * 512], BF16, tag=f"au{i}", name=f"au{i}")
                      for i in range(3)]

                def av_z(b, nj, aut, pav, zt):
                    # AV: col-tiled h-pair (concurrent), head-stacked output;
                    # Z: 2 concurrent M=1 col-tiled matmuls
                    for h in range(HL):
                        nc.tensor.matmul(
                            pav[b][h * DH:(h + 1) * DH, :],
                            va[b, nj][:, h * DH:(h + 1) * DH],
                            aut[:, h * 512:(h + 1) * 512],
                            start=(nj == 0), stop=(nj == njb - 1),
                            tile_position=(0, h * DH),
                        )
                    for h in range(HL):
                        s = b * HL + h
                        nc.tensor.matmul(
                            zt[s * 32:s * 32 + 1, :],
                            ones,
                            aut[:, h * 512:(h + 1) * 512],
                            start=(nj == 0), stop=(nj == njb - 1),
                            tile_position=(0, s * 32),
                        )

                def po_chunk(ni, c):
                    # output projection for one 512-dim half of a 128-token
                    # block of query chunk ni (interleaved into later rounds)
                    tb, half = c // 2, c % 2
                    b, r = tb // (512 // 128), tb % (512 // 128)
                    r0 = ni * 512 + r * 128
                    po = popp.tile([128, 512], F32, tag="po", name=f"po_{ni}_{c}")
                    nc.tensor.matmul(
                        po, av_n[b][:, r0:r0 + 128],
                        wo_full[:, half * 512:(half + 1) * 512],
                        start=True, stop=True,
                    )
                    os_ = osp.tile([128, 512], BF16, tag="os", name=f"os_{ni}_{c}")
                    nc.vector.tensor_copy(os_, po)
                    nc.gpsimd.dma_start(
                        out=out[b * n_sz + r0:b * n_sz + r0 + 128,
                                half * 512:(half + 1) * 512],
                        in_=os_)

                r = 0  # global round counter (one round = one batch's pair)
                for ni in range(nic):
                    pav = {b: pavp.tile([128, 512], F32, tag=f"pav{b}",
                                        name=f"pav_{ni}_{b}") for b in range(b_sz)}
                    zt = pzp.tile([128, 512], F32, tag="zt", name=f"zt_{ni}")
                    prev = None
                    for nj in range(njb):
                        # both heads' bias block in ONE DMA: [2,128,512] ->
                        # [128, 2, 512]; shared by both batches' rounds
                        bt2 = biasp.tile([128, HL, 512], BF16, tag="bt2",
                                         name=f"bt_{ni}_{nj}")
                        nc.sync.dma_start(
                            out=bt2,
                            in_=biasT[:, nj * 128:(nj + 1) * 128,
                                      ni * 512:(ni + 1) * 512]
                            .rearrange("h p q -> p h q"),
                        )
                        for b in range(b_sz):
                            sbuf = psim[r % 2]
                            aet, aut = ae[r % 3], au[r % 3]
                            # sims: the two heads go to different row groups
                            # AND different PSUM banks -> concurrent
                            for h in range(HL):
                                nc.tensor.matmul(
                                    sbuf[:, h * 512:(h + 1) * 512],
                                    kT[h * DH:(h + 1) * DH,
                                       b * n_sz + nj * 128:b * n_sz + (nj + 1) * 128],
                                    qT[h * DH:(h + 1) * DH,
                                       b * n_sz + ni * 512:b * n_sz + (ni + 1) * 512],
                                    start=True, stop=True,
                                )
                            # one exp spanning both heads' PSUM banks
                            nc.scalar.activation(out=aet, in_=sbuf, func=AF.Exp)
                            # bias multiply: ONE DVE op -> all four AV/Z
                            # matmuls become ready together and pack
                            nc.vector.tensor_mul(aut, aet,
                                                 bt2.rearrange("p h q -> p (h q)"))
                            # AV/Z for the PREVIOUS round (software pipeline)
                            if prev is not None:
                                av_z(prev[0], prev[1], au[prev[2] % 3], pav, zt)
                            prev = (b, nj, r)
                            r += 1
                        # interleave the previous query-chunk's output proj
                        if ni > 0:
                            po_chunk(ni - 1, nj)
                    av_z(prev[0], prev[1], au[prev[2] % 3], pav, zt)
                    # ---- ni boundary: evacuate pav, reciprocal Z, roundtrip ----
                    for b in range(b_sz):
                        nc.vector.tensor_copy(
                            av_u[b][:, ni * 512:(ni + 1) * 512], pav[b])
                    zrf = zc.tile([128, 512], F32, tag="zrf", name=f"zrf_{ni}")
                    nc.vector.reciprocal_approx_fast(zrf, zt)
                    zr = zc.tile([128, 512], BF16, tag="zr", name=f"zr_{ni}")
                    with nc.allow_low_precision(reason="1/Z bf16; ~4e-3 ok at 2e-2 gate"):
                        nc.vector.tensor_copy(zr, zrf)
                    for b in range(b_sz):
                        for h in range(HL):
                            s = b * HL + h
                            nc.gpsimd.dma_start(
                                out=zdram[b, h, 0, ni * 512:(ni + 1) * 512],
                                in_=zr[s * 32:s * 32 + 1, :])
                    for b in range(b_sz):
                        zb = zc.tile([128, 512], BF16, tag="zbb", name=f"zbb_{ni}_{b}")
                        for h in range(HL):
                            nc.gpsimd.dma_start(
                                out=zb[h * DH:(h + 1) * DH, :],
                                in_=zdram[b, h, :, ni * 512:(ni + 1) * 512]
                                .to_broadcast([DH, 512]))
                        with nc.allow_low_precision(reason="bf16 attention weights"):
                            nc.vector.tensor_mul(
                                av_n[b][:, ni * 512:(ni + 1) * 512],
                                av_u[b][:, ni * 512:(ni + 1) * 512],
                                zb)
            # ---- epilogue: last query-chunk's output projection ----
            with (
                tc.tile_pool(name="pout", bufs=4, space="PSUM") as pout,
                tc.tile_pool(name="ost2", bufs=6) as ost2,
            ):
                ni = nic - 1
                for c in range(2 * (512 // 128) * b_sz):
                    tb, half = c // 2, c % 2
                    b, rr = tb // (512 // 128), tb % (512 // 128)
                    r0 = ni * 512 + rr * 128
                    po = pout.tile([128, 512], F32, tag="po", name=f"poe_{c}")
                    nc.tensor.matmul(
                        po, av_n[b][:, r0:r0 + 128],
                        wo_full[:, half * 512:(half + 1) * 512],
                        start=True, stop=True,
                    )
                    os_ = ost2.tile([128, 512], BF16, tag="os", name=f"ose_{c}")
                    if c % 2 == 0:
                        nc.vector.tensor_copy(os_, po)
                    else:
                        nc.scalar.activation(out=os_, in_=po, func=AF.Copy)
                    nc.sync.dma_start(
                        out=out[b * n_sz + r0:b * n_sz + r0 + 128,
                                half * 512:(half + 1) * 512],
                        in_=os_)
    nc.compile()
    return nc


_NC_CACHE = {}


def _get_nc(key, **kw):
    if key not in _NC_CACHE:
        _NC_CACHE[key] = build(**kw)
    return _NC_CACHE[key]


def make_in_maps(x, rel_pos_bias, g, Wq, Wkv, Wo):
    b_sz, n_sz, dim = x.shape
    inner = Wq.shape[0]
    x = np.asarray(x, np.float32)
    xTh = np.ascontiguousarray(x.transpose(0, 2, 1)).astype(BF)  # [B, DIM, N]
    gv = np.asarray(g, np.float32).reshape(1, dim)
    Wq = np.asarray(Wq, np.float32) * gv
    Wkv = np.asarray(Wkv, np.float32) * gv
    scale = DH ** -0.5
    in_maps = []
    for c in range(NCORES):
        rs, re = c * IL, (c + 1) * IL
        wq_c = np.ascontiguousarray((Wq[rs:re, :] * scale).T).astype(BF)
        wk_c = np.ascontiguousarray(Wkv[rs:re, :].T).astype(BF)
        wv_c = np.ascontiguousarray(Wkv[inner + rs:inner + re, :].T).astype(BF)
        wo_c = np.ascontiguousarray(np.asarray(Wo)[:, rs:re].T).astype(BF)
        bias_c = np.exp(np.ascontiguousarray(
            np.asarray(rel_pos_bias)[0, c * HL:(c + 1) * HL].transpose(0, 2, 1)
        )).astype(BF)
        in_maps.append({
            "xT": xTh, "wqT": wq_c, "wkT": wk_c, "wvT": wv_c,
            "woT": wo_c, "biasT": bias_c,
        })
    return in_maps


def kernel(x, rel_pos_bias, g, Wq, Wkv, Wo):
    b_sz, n_sz, dim = x.shape
    nc = _get_nc((b_sz, n_sz, dim), b_sz=b_sz, n_sz=n_sz, dim=dim)
    in_maps = make_in_maps(x, rel_pos_bias, g, Wq, Wkv, Wo)
    res = run_bass_kernel_spmd(nc, in_maps, core_ids=list(range(NCORES)))
    acc = np.zeros((b_sz * n_sz, dim), np.float32)
    for r in res.results:
        acc += np.asarray(r["out"]).astype(np.float32)
    return np.ascontiguousarray(acc.reshape(b_sz, n_sz, dim))


# revision 16
# speedup vs baseline: 1.0736x; 1.0736x over previous
"""Trainium2 Bass kernel: multi-head attention with sequence-axis layernorm
and relative position bias, sharded 8-way over heads (2 heads/core).

v2 layout strategy (per core):
  - LN over sequence axis in [d_partition, n_free] layout; stats on DVE
    (bn_stats/bn_aggr), apply on DVE via fused tensor_scalar (x*scl + nshf)
    in bf16 (4x mode); g folded into Wq/Wkv on the host.
  - qT/kT [inner_local=128, b*n] via const-weight matmuls (K=128, Nf=512).
  - v natural per (b, nj): va_full[b,nj] [128 tokens, 128 inner] bf16.
  - attention rounds (ni, nj): all 4 streams (b x h) share ONE persistent
    4-bank PSUM tile [128, 2048] f32, cols [b0h0|b0h1|b1h0|b1h1]; the two
    sims of a batch are row-tiled (K=64 at row groups 0/64) into DIFFERENT
    banks and run concurrently on the PE.
  - exp: ONE ScalarE activation per batch-pair [128, 1024] spanning 2 PSUM
    banks (amortizes the ~352-cycle ACT instruction overhead).
  - bias folded multiplicatively: host precomputes exp(biasT) bf16; DVE
    multiplies (2x mode) into au.
  - AV: col-tiled pairs — h0 -> pav[b][0:64], h1 -> pav[b][64:128] (M=64,
    col groups disjoint -> concurrent), accumulated over nj. This yields a
    head-STACKED av [128, qi] enabling a K=128 output projection.
  - Z: separate [128,512] PSUM bank; 4 concurrent M=1 col-tiled matmuls
    with ones-weights at partitions {0,32,64,96}, accumulated over nj.
  - softmax denominator: reciprocal on DVE at ni boundary, DRAM roundtrip
    to broadcast 1/Z rows across 64 partitions; normalization deferred to
    the out-projection phase (off the attention critical path).
  - out-proj: stacked K=128 matmuls (lhsT = av_n [128, tok]), PSUM->SBUF
    copies alternating DVE/ACT, bf16 partial output summed on host in f32.
"""

import numpy as np
import ml_dtypes

import concourse.bass as bass
from concourse import bacc
import concourse.mybir as mybir
import concourse.tile as tile
from concourse.bass_utils import run_bass_kernel_spmd

F32 = mybir.dt.float32
BF16 = mybir.dt.bfloat16
BF = ml_dtypes.bfloat16
AF = mybir.ActivationFunctionType
ALU = mybir.AluOpType

# full-size problem constants
B, N, DIM = 2, 2048, 1024
HEADS, DH = 16, 64
NCORES = 8
HL = HEADS // NCORES          # heads per core = 2
IL = HL * DH                  # local inner = 128
INNER = HEADS * DH            # 1024


def build(b_sz=B, n_sz=N, dim=DIM, eps=1e-5):
    """Build the per-core Bass graph (SPMD across 8 cores)."""
    nd = dim // 128               # d tiles
    nch = (b_sz * n_sz) // 512    # 512-col chunks of flattened b*n
    njb = n_sz // 128             # key tiles per batch
    nic = n_sz // 512             # query chunks per batch
    bn = b_sz * n_sz
    nsub = n_sz // 512            # bn_stats subgroups

    nc = bacc.Bacc(None, target_bir_lowering=False)
    xT = nc.declare_dram_parameter("xT", [b_sz, dim, n_sz], BF16, isOutput=False)
    wqT = nc.declare_dram_parameter("wqT", [dim, IL], BF16, isOutput=False)
    wkT = nc.declare_dram_parameter("wkT", [dim, IL], BF16, isOutput=False)
    wvT = nc.declare_dram_parameter("wvT", [dim, IL], BF16, isOutput=False)
    woT = nc.declare_dram_parameter("woT", [IL, dim], BF16, isOutput=False)
    biasT = nc.declare_dram_parameter("biasT", [HL, n_sz, n_sz], BF16, isOutput=False)  # exp(bias.T)
    out = nc.declare_dram_parameter("out", [bn, dim], BF16, isOutput=True)
    zdram = nc.dram_tensor("zscratch", [b_sz, HL, 1, n_sz], BF16)

    with tile.TileContext(nc) as tc:
        with (
            tc.tile_pool(name="consts", bufs=1) as consts,
            tc.tile_pool(name="persist", bufs=1) as persist,
        ):
            # ---- load weights ----
            wq_s, wk_s, wv_s = [], [], []
            for dt in range(nd):
                for lst, src, nm in ((wq_s, wqT, "wq"), (wk_s, wkT, "wk"), (wv_s, wvT, "wv")):
                    t = consts.tile([128, IL], BF16, tag=f"{nm}{dt}")
                    nc.sync.dma_start(out=t, in_=src[dt * 128:(dt + 1) * 128, :])
                    lst.append(t)
            wo_full = consts.tile([IL, dim], BF16, tag="wo")
            nc.sync.dma_start(out=wo_full, in_=woT[:, :])
            ones = consts.tile([128, 1], BF16, tag="ones")
            nc.vector.memset(ones, 1.0)

            xn = {}
            qT = persist.tile([IL, bn], BF16, tag="qT")
            kT = persist.tile([IL, bn], BF16, tag="kT")
            va = {}   # (b, nj) -> [128 tokens, 128 inner] bf16

            # ---- Phase 1: layernorm over sequence axis ----
            # mean/var via DVE bn_stats for half the tiles, via ScalarE
            # Square/Identity+accum_out for the other half (the two engines
            # run the stats concurrently; DVE was the pre-phase bottleneck)
            inv_n = 1.0 / n_sz
            with (
                tc.tile_pool(name="xload", bufs=4) as xload,
                tc.tile_pool(name="lns", bufs=8) as lns,
                tc.tile_pool(name="lnscr", bufs=2) as lnscr,
            ):
                for b in range(b_sz):
                    for dt in range(nd):
                        xt = xload.tile([128, n_sz], BF16, tag="xt",
                                        name=f"xt_{b}_{dt}")
                        nc.sync.dma_start(out=xt, in_=xT[b, dt * 128:(dt + 1) * 128, :])
                        mv = lns.tile([128, 2], F32, tag="mv", name=f"mv_{b}_{dt}")
                        if (b * nd + dt) % 2 == 0:
                            stats = lns.tile([128, nsub, 6], F32, tag="stats",
                                             name=f"st_{b}_{dt}")
                            for s in range(nsub):
                                nc.vector.bn_stats(out=stats[:, s, :],
                                                   in_=xt[:, s * 512:(s + 1) * 512])
                            nc.vector.bn_aggr(out=mv, in_=stats)
                        else:
                            scr = lnscr.tile([128, n_sz], BF16, tag="scr",
                                             name=f"scr_{b}_{dt}")
                            sums = lns.tile([128, 2], F32, tag="sums",
                                            name=f"sums_{b}_{dt}")
                            nc.scalar.activation(out=scr, in_=xt, func=AF.Identity,
                                                 accum_out=sums[:, 0:1])
                            nc.scalar.activation(out=scr, in_=xt, func=AF.Square,
                                                 accum_out=sums[:, 1:2])
                            # mean = sum/n ; var = sumsq/n - mean^2
                            nc.vector.tensor_scalar_mul(mv[:, 0:1], sums[:, 0:1], inv_n)
                            msq = lns.tile([128, 1], F32, tag="msq",
                                           name=f"msq_{b}_{dt}")
                            nc.vector.tensor_mul(msq, mv[:, 0:1], mv[:, 0:1])
                            with nc.allow_low_precision(reason="var f32"):
                                nc.vector.tensor_scalar(
                                    mv[:, 1:2], sums[:, 1:2], inv_n, msq,
                                    ALU.mult, ALU.subtract)
                        vmax = lns.tile([128, 1], F32, tag="vmax", name=f"vm_{b}_{dt}")
                        nc.vector.tensor_scalar_max(vmax, mv[:, 1:2], eps)
                        sq = lns.tile([128, 1], F32, tag="sq", name=f"sq_{b}_{dt}")
                        nc.scalar.activation(out=sq, in_=vmax, func=AF.Sqrt)
                        scl = lns.tile([128, 1], F32, tag="scl", name=f"scl_{b}_{dt}")
                        nc.vector.reciprocal(scl, sq)
                        nshf = lns.tile([128, 1], F32, tag="nshf", name=f"ns_{b}_{dt}")
                        with nc.allow_low_precision(reason="mean*scl in f32; fine"):
                            nc.vector.tensor_scalar(
                                nshf, mv[:, 0:1], scl, -1.0, ALU.mult, ALU.mult)
                        xnt = persist.tile([128, n_sz], BF16, tag=f"xn_{b}_{dt}")
                        with nc.allow_low_precision(reason="bf16 LN apply; ~4e-3 ok"):
                            nc.vector.tensor_scalar(
                                xnt, xt, scl, nshf, ALU.mult, ALU.add)
                        xn[b, dt] = xnt

            # ---- Phase 2a: q/k projections (transposed layout) ----
            with tc.tile_pool(name="pproj", bufs=4, space="PSUM") as pproj:
                for ch in range(nch):
                    b = (ch * 512) // n_sz
                    col0 = (ch * 512) % n_sz
                    for (w_s, dst) in ((wq_s, qT), (wk_s, kT)):
                        ps = pproj.tile([IL, 512], F32, tag="ps")
                        for dt in range(nd):
                            nc.tensor.matmul(
                                ps, w_s[dt], xn[b, dt][:, col0:col0 + 512],
                                start=(dt == 0), stop=(dt == nd - 1),
                            )
                        nc.scalar.activation(out=dst[:, ch * 512:(ch + 1) * 512],
                                             in_=ps, func=AF.Copy)

            # ---- Phase 2b: v natural [tokens, inner] ----
            with tc.tile_pool(name="pv", bufs=4, space="PSUM") as pv:
                for b in range(b_sz):
                    for nj in range(njb):
                        psv = pv.tile([128, IL], F32, tag="psv", name=f"psv_{b}_{nj}")
                        for dt in range(nd):
                            nc.tensor.matmul(
                                psv, xn[b, dt][:, nj * 128:(nj + 1) * 128], wv_s[dt],
                                start=(dt == 0), stop=(dt == nd - 1),
                            )
                        t = persist.tile([128, IL], BF16, tag=f"va_{b}_{nj}")
                        nc.scalar.activation(out=t, in_=psv, func=AF.Copy)
                        va[b, nj] = t

            # ---- Phase 3: attention ----
            # stream -> psim column range: [b0h0 | b0h1 | b1h0 | b1h1]
            # (each 512 f32 = exactly one PSUM bank; b-pairs adjacent so one
            # 1024-wide exp covers both heads of a batch)
            av_u = {b: persist.tile([128, n_sz], BF16, tag=f"avu_{b}",
                                    name=f"avu_{b}")
                    for b in range(b_sz)}
            av_n = {b: persist.tile([128, n_sz], BF16, tag=f"avn_{b}",
                                    name=f"avn_{b}")
                    for b in range(b_sz)}
            zbb = {}  # (b, ni) -> [128, 512] bf16 stacked 1/Z broadcast
            with (
                tc.tile_pool(name="psim", bufs=1, space="PSUM") as psimp,
                tc.tile_pool(name="pav", bufs=1, space="PSUM") as pavp,
                tc.tile_pool(name="pz", bufs=1, space="PSUM") as pzp,
                tc.tile_pool(name="pop", bufs=1, space="PSUM") as popp,
                tc.tile_pool(name="biasp", bufs=4) as biasp,
                tc.tile_pool(name="aep", bufs=1) as aep,
                tc.tile_pool(name="osp", bufs=4) as osp,
                tc.tile_pool(name="zc", bufs=4) as zc,
            ):
                # two 2-bank sim buffers: exp(r) reads one while the next
                # round's sims fill the other (the exp pipeline never stalls)
                psim = [psimp.tile([128, HL * 512], F32, tag=f"psim{i}",
                                   name=f"psim{i}") for i in range(2)]
                ae = [aep.tile([128, HL * 512], BF16, tag=f"ae{i}", name=f"ae{i}")
                      for i in range(3)]
                au = [aep.tile([128, HL * 512], BF16, tag=f"au{i}", name=f"au{i}")
                      for i in range(3)]

                def av_z(b, nj, aut, pav, zt):
                    # AV: col-tiled h-pair (concurrent), head-stacked output;
                    # Z: 2 concurrent M=1 col-tiled matmuls
                    for h in range(HL):
                        nc.tensor.matmul(
                            pav[b][h * DH:(h + 1) * DH, :],
                            va[b, nj][:, h * DH:(h + 1) * DH],
                            aut[:, h * 512:(h + 1) * 512],
                            start=(nj == 0), stop=(nj == njb - 1),
                            tile_position=(0, h * DH),
                        )
                    for h in range(HL):
                        s = b * HL + h
                        nc.tensor.matmul(
                            zt[s * 32:s * 32 + 1, :],
                            ones,
                            aut[:, h * 512:(h + 1) * 512],
                            start=(nj == 0), stop=(nj == njb - 1),
                            tile_position=(0, s * 32),
                        )

                def po_chunk(ni, c):
                    # output projection for one 512-dim half of a 128-token
                    # block of query chunk ni (interleaved into later rounds)
                    tb, half = c // 2, c % 2
                    b, r = tb // (512 // 128), tb % (512 // 128)
                    r0 = ni * 512 + r * 128
                    po = popp.tile([128, 512], F32, tag="po", name=f"po_{ni}_{c}")
                    nc.tensor.matmul(
                        po, av_n[b][:, r0:r0 + 128],
                        wo_full[:, half * 512:(half + 1) * 512],
                        start=True, stop=True,
                    )
                    os_ = osp.tile([128, 512], BF16, tag="os", name=f"os_{ni}_{c}")
                    nc.vector.tensor_copy(os_, po)
                    nc.sync.dma_start(
                        out=out[b * n_sz + r0:b * n_sz + r0 + 128,
                                half * 512:(half + 1) * 512],
                        in_=os_)

                r = 0  # global round counter (one round = one batch's pair)
                for ni in range(nic):
                    pav = {b: pavp.tile([128, 512], F32, tag=f"pav{b}",
                                        name=f"pav_{ni}_{b}") for b in range(b_sz)}
                    zt = pzp.tile([128, 512], F32, tag="zt", name=f"zt_{ni}")
                    prev = None
                    for nj in range(njb):
                        # both heads' bias block in ONE DMA: [2,128,512] ->
                        # [128, 2, 512]; shared by both batches' rounds
                        bt2 = biasp.tile([128, HL, 512], BF16, tag="bt2",
                                         name=f"bt_{ni}_{nj}")
                        nc.sync.dma_start(
                            out=bt2,
                            in_=biasT[:, nj * 128:(nj + 1) * 128,
                                      ni * 512:(ni + 1) * 512]
                            .rearrange("h p q -> p h q"),
                        )
                        for b in range(b_sz):
                            sbuf = psim[r % 2]
                            aet, aut = ae[r % 3], au[r % 3]
                            # sims: the two heads go to different row groups
                            # AND different PSUM banks -> concurrent
                            for h in range(HL):
                                nc.tensor.matmul(
                                    sbuf[:, h * 512:(h + 1) * 512],
                                    kT[h * DH:(h + 1) * DH,
                                       b * n_sz + nj * 128:b * n_sz + (nj + 1) * 128],
                                    qT[h * DH:(h + 1) * DH,
                                       b * n_sz + ni * 512:b * n_sz + (ni + 1) * 512],
                                    start=True, stop=True,
                                )
                            # one exp spanning both heads' PSUM banks
                            nc.scalar.activation(out=aet, in_=sbuf, func=AF.Exp)
                            # bias multiply: ONE DVE op -> all four AV/Z
                            # matmuls become ready together and pack
                            nc.vector.tensor_mul(aut, aet, bt2)
                            # AV/Z for the PREVIOUS round (software pipeline)
                            if prev is not None:
                                av_z(prev[0], prev[1], au[prev[2] % 3], pav, zt)
                            prev = (b, nj, r)
                            r += 1
                        # interleave the previous query-chunk's output proj
                        if ni > 0:
                            po_chunk(ni - 1, nj)
                    av_z(prev[0], prev[1], au[prev[2] % 3], pav, zt)
                    # ---- ni boundary: evacuate pav, reciprocal Z, roundtrip ----
                    for b in range(b_sz):
                        nc.vector.tensor_copy(
                            av_u[b][:, ni * 512:(ni + 1) * 512], pav[b])
                    zrf = zc.tile([128, 512], F32, tag="zrf", name=f"zrf_{ni}")
                    nc.vector.reciprocal_approx_fast(zrf, zt)
                    zr = zc.tile([128, 512], BF16, tag="zr", name=f"zr_{ni}")
                    with nc.allow_low_precision(reason="1/Z bf16; ~4e-3 ok at 2e-2 gate"):
                        nc.vector.tensor_copy(zr, zrf)
                    for b in range(b_sz):
                        for h in range(HL):
                            s = b * HL + h
                            nc.sync.dma_start(
                                out=zdram[b, h, 0, ni * 512:(ni + 1) * 512],
                                in_=zr[s * 32:s * 32 + 1, :])
                    for b in range(b_sz):
                        zb = zc.tile([128, 512], BF16, tag="zbb", name=f"zbb_{ni}_{b}")
                        for h in range(HL):
                            nc.sync.dma_start(
                                out=zb[h * DH:(h + 1) * DH, :],
                                in_=zdram[b, h, :, ni * 512:(ni + 1) * 512]
                                .to_broadcast([DH, 512]))
                        with nc.allow_low_precision(reason="bf16 attention weights"):
                            nc.vector.tensor_mul(
                                av_n[b][:, ni * 512:(ni + 1) * 512],
                                av_u[b][:, ni * 512:(ni + 1) * 512],
                                zb)
            # ---- epilogue: last query-chunk's output projection ----
            with (
                tc.tile_pool(name="pout", bufs=4, space="PSUM") as pout,
                tc.tile_pool(name="ost2", bufs=6) as ost2,
            ):
                ni = nic - 1
                for c in range(2 * (512 // 128) * b_sz):
                    tb, half = c // 2, c % 2
                    b, rr = tb // (512 // 128), tb % (512 // 128)
                    r0 = ni * 512 + rr * 128
                    po = pout.tile([128, 512], F32, tag="po", name=f"poe_{c}")
                    nc.tensor.matmul(
                        po, av_n[b][:, r0:r0 + 128],
                        wo_full[:, half * 512:(half + 1) * 512],
                        start=True, stop=True,
                    )
                    os_ = ost2.tile([128, 512], BF16, tag="os", name=f"ose_{c}")
                    if c % 2 == 0:
                        nc.vector.tensor_copy(os_, po)
                    else:
                        nc.scalar.activation(out=os_, in_=po, func=AF.Copy)
                    nc.sync.dma_start(
                        out=out[b * n_sz + r0:b * n_sz + r0 + 128,
                                half * 512:(half + 1) * 512],
                        in_=os_)
    nc.compile()
    return nc


_NC_CACHE = {}


def _get_nc(key, **kw):
    if key not in _NC_CACHE:
        _NC_CACHE[key] = build(**kw)
    return _NC_CACHE[key]


def make_in_maps(x, rel_pos_bias, g, Wq, Wkv, Wo):
    b_sz, n_sz, dim = x.shape
    inner = Wq.shape[0]
    x = np.asarray(x, np.float32)
    xTh = np.ascontiguousarray(x.transpose(0, 2, 1)).astype(BF)  # [B, DIM, N]
    gv = np.asarray(g, np.float32).reshape(1, dim)
    Wq = np.asarray(Wq, np.float32) * gv
    Wkv = np.asarray(Wkv, np.float32) * gv
    scale = DH ** -0.5
    in_maps = []
    for c in range(NCORES):
        rs, re = c * IL, (c + 1) * IL
        wq_c = np.ascontiguousarray((Wq[rs:re, :] * scale).T).astype(BF)
        wk_c = np.ascontiguousarray(Wkv[rs:re, :].T).astype(BF)
        wv_c = np.ascontiguousarray(Wkv[inner + rs:inner + re, :].T).astype(BF)
        wo_c = np.ascontiguousarray(np.asarray(Wo)[:, rs:re].T).astype(BF)
        bias_c = np.exp(np.ascontiguousarray(
            np.asarray(rel_pos_bias)[0, c * HL:(c + 1) * HL].transpose(0, 2, 1)
        )).astype(BF)
        in_maps.append({
            "xT": xTh, "wqT": wq_c, "wkT": wk_c, "wvT": wv_c,
            "woT": wo_c, "biasT": bias_c,
        })
    return in_maps


def kernel(x, rel_pos_bias, g, Wq, Wkv, Wo):
    b_sz, n_sz, dim = x.shape
    nc = _get_nc((b_sz, n_sz, dim), b_sz=b_sz, n_sz=n_sz, dim=dim)
    in_maps = make_in_maps(x, rel_pos_bias, g, Wq, Wkv, Wo)
    res = run_bass_kernel_spmd(nc, in_maps, core_ids=list(range(NCORES)))
    acc = np.zeros((b_sz * n_sz, dim), np.float32)
    for r in res.results:
        acc += np.asarray(r["out"]).astype(np.float32)
    return np.ascontiguousarray(acc.reshape(b_sz, n_sz, dim))


# revision 17
# speedup vs baseline: 1.2257x; 1.1416x over previous
"""Trainium2 Bass kernel: multi-head attention with sequence-axis layernorm
and relative position bias, sharded 8-way over heads (2 heads/core).

v2 layout strategy (per core):
  - LN over sequence axis in [d_partition, n_free] layout; stats on DVE
    (bn_stats/bn_aggr), apply on DVE via fused tensor_scalar (x*scl + nshf)
    in bf16 (4x mode); g folded into Wq/Wkv on the host.
  - qT/kT [inner_local=128, b*n] via const-weight matmuls (K=128, Nf=512).
  - v natural per (b, nj): va_full[b,nj] [128 tokens, 128 inner] bf16.
  - attention rounds (ni, nj): all 4 streams (b x h) share ONE persistent
    4-bank PSUM tile [128, 2048] f32, cols [b0h0|b0h1|b1h0|b1h1]; the two
    sims of a batch are row-tiled (K=64 at row groups 0/64) into DIFFERENT
    banks and run concurrently on the PE.
  - exp: ONE ScalarE activation per batch-pair [128, 1024] spanning 2 PSUM
    banks (amortizes the ~352-cycle ACT instruction overhead).
  - bias folded multiplicatively: host precomputes exp(biasT) bf16; DVE
    multiplies (2x mode) into au.
  - AV: col-tiled pairs — h0 -> pav[b][0:64], h1 -> pav[b][64:128] (M=64,
    col groups disjoint -> concurrent), accumulated over nj. This yields a
    head-STACKED av [128, qi] enabling a K=128 output projection.
  - Z: separate [128,512] PSUM bank; 4 concurrent M=1 col-tiled matmuls
    with ones-weights at partitions {0,32,64,96}, accumulated over nj.
  - softmax denominator: reciprocal on DVE at ni boundary, DRAM roundtrip
    to broadcast 1/Z rows across 64 partitions; normalization deferred to
    the out-projection phase (off the attention critical path).
  - out-proj: stacked K=128 matmuls (lhsT = av_n [128, tok]), PSUM->SBUF
    copies alternating DVE/ACT, bf16 partial output summed on host in f32.
"""

import numpy as np
import ml_dtypes

import concourse.bass as bass
from concourse import bacc
import concourse.mybir as mybir
import concourse.tile as tile
from concourse.bass_utils import run_bass_kernel_spmd

F32 = mybir.dt.float32
BF16 = mybir.dt.bfloat16
BF = ml_dtypes.bfloat16
AF = mybir.ActivationFunctionType
ALU = mybir.AluOpType

# full-size problem constants
B, N, DIM = 2, 2048, 1024
HEADS, DH = 16, 64
NCORES = 8
HL = HEADS // NCORES          # heads per core = 2
IL = HL * DH                  # local inner = 128
INNER = HEADS * DH            # 1024


def build(b_sz=B, n_sz=N, dim=DIM, eps=1e-5):
    """Build the per-core Bass graph (SPMD across 8 cores)."""
    nd = dim // 128               # d tiles
    nch = (b_sz * n_sz) // 512    # 512-col chunks of flattened b*n
    njb = n_sz // 128             # key tiles per batch
    nic = n_sz // 512             # query chunks per batch
    bn = b_sz * n_sz
    nsub = n_sz // 512            # bn_stats subgroups

    nc = bacc.Bacc(None, target_bir_lowering=False)
    xT = nc.declare_dram_parameter("xT", [b_sz, dim, n_sz], BF16, isOutput=False)
    wqT = nc.declare_dram_parameter("wqT", [dim, IL], BF16, isOutput=False)
    wkT = nc.declare_dram_parameter("wkT", [dim, IL], BF16, isOutput=False)
    wvT = nc.declare_dram_parameter("wvT", [dim, IL], BF16, isOutput=False)
    woT = nc.declare_dram_parameter("woT", [IL, dim], BF16, isOutput=False)
    biasT = nc.declare_dram_parameter("biasT", [HL, n_sz, n_sz], BF16, isOutput=False)  # exp(bias.T)
    out = nc.declare_dram_parameter("out", [bn, dim], BF16, isOutput=True)
    zdram = nc.dram_tensor("zscratch", [b_sz, HL, 1, n_sz], BF16)

    with tile.TileContext(nc) as tc:
        with (
            tc.tile_pool(name="consts", bufs=1) as consts,
            tc.tile_pool(name="persist", bufs=1) as persist,
        ):
            # ---- load weights ----
            wq_s, wk_s, wv_s = [], [], []
            for dt in range(nd):
                for lst, src, nm in ((wq_s, wqT, "wq"), (wk_s, wkT, "wk"), (wv_s, wvT, "wv")):
                    t = consts.tile([128, IL], BF16, tag=f"{nm}{dt}")
                    nc.sync.dma_start(out=t, in_=src[dt * 128:(dt + 1) * 128, :])
                    lst.append(t)
            wo_full = consts.tile([IL, dim], BF16, tag="wo")
            nc.sync.dma_start(out=wo_full, in_=woT[:, :])
            ones = consts.tile([128, 1], BF16, tag="ones")
            nc.vector.memset(ones, 1.0)

            xn = {}
            qT = persist.tile([IL, bn], BF16, tag="qT")
            kT = persist.tile([IL, bn], BF16, tag="kT")
            va = {}   # (b, nj) -> [128 tokens, 128 inner] bf16

            # ---- Phase 1: layernorm over sequence axis ----
            # mean/var via DVE bn_stats for half the tiles, via ScalarE
            # Square/Identity+accum_out for the other half (the two engines
            # run the stats concurrently; DVE was the pre-phase bottleneck)
            inv_n = 1.0 / n_sz
            with (
                tc.tile_pool(name="xload", bufs=4) as xload,
                tc.tile_pool(name="lns", bufs=8) as lns,
                tc.tile_pool(name="lnscr", bufs=2) as lnscr,
            ):
                for b in range(b_sz):
                    for dt in range(nd):
                        xt = xload.tile([128, n_sz], BF16, tag="xt",
                                        name=f"xt_{b}_{dt}")
                        nc.sync.dma_start(out=xt, in_=xT[b, dt * 128:(dt + 1) * 128, :])
                        mv = lns.tile([128, 2], F32, tag="mv", name=f"mv_{b}_{dt}")
                        if (b * nd + dt) % 2 == 0:
                            stats = lns.tile([128, nsub, 6], F32, tag="stats",
                                             name=f"st_{b}_{dt}")
                            for s in range(nsub):
                                nc.vector.bn_stats(out=stats[:, s, :],
                                                   in_=xt[:, s * 512:(s + 1) * 512])
                            nc.vector.bn_aggr(out=mv, in_=stats)
                        else:
                            scr = lnscr.tile([128, n_sz], BF16, tag="scr",
                                             name=f"scr_{b}_{dt}")
                            sums = lns.tile([128, 2], F32, tag="sums",
                                            name=f"sums_{b}_{dt}")
                            nc.scalar.activation(out=scr, in_=xt, func=AF.Identity,
                                                 accum_out=sums[:, 0:1])
                            nc.scalar.activation(out=scr, in_=xt, func=AF.Square,
                                                 accum_out=sums[:, 1:2])
                            # mean = sum/n ; var = sumsq/n - mean^2
                            nc.vector.tensor_scalar_mul(mv[:, 0:1], sums[:, 0:1], inv_n)
                            msq = lns.tile([128, 1], F32, tag="msq",
                                           name=f"msq_{b}_{dt}")
                            nc.vector.tensor_mul(msq, mv[:, 0:1], mv[:, 0:1])
                            with nc.allow_low_precision(reason="var f32"):
                                nc.vector.tensor_scalar(
                                    mv[:, 1:2], sums[:, 1:2], inv_n, msq,
                                    ALU.mult, ALU.subtract)
                        vmax = lns.tile([128, 1], F32, tag="vmax", name=f"vm_{b}_{dt}")
                        nc.vector.tensor_scalar_max(vmax, mv[:, 1:2], eps)
                        sq = lns.tile([128, 1], F32, tag="sq", name=f"sq_{b}_{dt}")
                        nc.scalar.activation(out=sq, in_=vmax, func=AF.Sqrt)
                        scl = lns.tile([128, 1], F32, tag="scl", name=f"scl_{b}_{dt}")
                        nc.vector.reciprocal(scl, sq)
                        nshf = lns.tile([128, 1], F32, tag="nshf", name=f"ns_{b}_{dt}")
                        with nc.allow_low_precision(reason="mean*scl in f32; fine"):
                            nc.vector.tensor_scalar(
                                nshf, mv[:, 0:1], scl, -1.0, ALU.mult, ALU.mult)
                        xnt = persist.tile([128, n_sz], BF16, tag=f"xn_{b}_{dt}")
                        with nc.allow_low_precision(reason="bf16 LN apply; ~4e-3 ok"):
                            nc.vector.tensor_scalar(
                                xnt, xt, scl, nshf, ALU.mult, ALU.add)
                        xn[b, dt] = xnt

            # ---- Phase 2a: q/k projections (transposed layout) ----
            with tc.tile_pool(name="pproj", bufs=4, space="PSUM") as pproj:
                for ch in range(nch):
                    b = (ch * 512) // n_sz
                    col0 = (ch * 512) % n_sz
                    for (w_s, dst) in ((wq_s, qT), (wk_s, kT)):
                        ps = pproj.tile([IL, 512], F32, tag="ps")
                        for dt in range(nd):
                            nc.tensor.matmul(
                                ps, w_s[dt], xn[b, dt][:, col0:col0 + 512],
                                start=(dt == 0), stop=(dt == nd - 1),
                            )
                        nc.scalar.activation(out=dst[:, ch * 512:(ch + 1) * 512],
                                             in_=ps, func=AF.Copy)

            # ---- Phase 2b: v natural [tokens, inner] ----
            with tc.tile_pool(name="pv", bufs=4, space="PSUM") as pv:
                for b in range(b_sz):
                    for nj in range(njb):
                        psv = pv.tile([128, IL], F32, tag="psv", name=f"psv_{b}_{nj}")
                        for dt in range(nd):
                            nc.tensor.matmul(
                                psv, xn[b, dt][:, nj * 128:(nj + 1) * 128], wv_s[dt],
                                start=(dt == 0), stop=(dt == nd - 1),
                            )
                        t = persist.tile([128, IL], BF16, tag=f"va_{b}_{nj}")
                        nc.scalar.activation(out=t, in_=psv, func=AF.Copy)
                        va[b, nj] = t

            # ---- Phase 3: attention ----
            # stream -> psim column range: [b0h0 | b0h1 | b1h0 | b1h1]
            # (each 512 f32 = exactly one PSUM bank; b-pairs adjacent so one
            # 1024-wide exp covers both heads of a batch)
            av_u = {b: persist.tile([128, n_sz], BF16, tag=f"avu_{b}",
                                    name=f"avu_{b}")
                    for b in range(b_sz)}
            av_n = {b: persist.tile([128, n_sz], BF16, tag=f"avn_{b}",
                                    name=f"avn_{b}")
                    for b in range(b_sz)}
            zbb = {}  # (b, ni) -> [128, 512] bf16 stacked 1/Z broadcast
            with (
                tc.tile_pool(name="psim", bufs=1, space="PSUM") as psimp,
                tc.tile_pool(name="pav", bufs=1, space="PSUM") as pavp,
                tc.tile_pool(name="pz", bufs=1, space="PSUM") as pzp,
                tc.tile_pool(name="pop", bufs=1, space="PSUM") as popp,
                tc.tile_pool(name="biasp", bufs=16) as biasp,
                tc.tile_pool(name="aep", bufs=1) as aep,
                tc.tile_pool(name="osp", bufs=4) as osp,
                tc.tile_pool(name="zc", bufs=4) as zc,
            ):
                # two 2-bank sim buffers: exp(r) reads one while the next
                # round's sims fill the other (the exp pipeline never stalls)
                psim = [psimp.tile([128, HL * 512], F32, tag=f"psim{i}",
                                   name=f"psim{i}") for i in range(2)]
                ae = [aep.tile([128, HL * 512], BF16, tag=f"ae{i}", name=f"ae{i}")
                      for i in range(3)]
                au = [aep.tile([128, HL * 512], BF16, tag=f"au{i}", name=f"au{i}")
                      for i in range(3)]

                def av_z(b, nj, aut, pav, zt):
                    # AV: col-tiled h-pair (concurrent), head-stacked output;
                    # Z: 2 concurrent M=1 col-tiled matmuls
                    for h in range(HL):
                        nc.tensor.matmul(
                            pav[b][h * DH:(h + 1) * DH, :],
                            va[b, nj][:, h * DH:(h + 1) * DH],
                            aut[:, h * 512:(h + 1) * 512],
                            start=(nj == 0), stop=(nj == njb - 1),
                            tile_position=(0, h * DH),
                        )
                    for h in range(HL):
                        s = b * HL + h
                        nc.tensor.matmul(
                            zt[s * 32:s * 32 + 1, :],
                            ones,
                            aut[:, h * 512:(h + 1) * 512],
                            start=(nj == 0), stop=(nj == njb - 1),
                            tile_position=(0, s * 32),
                        )

                def po_chunk(ni, c):
                    # output projection for one 512-dim half of a 128-token
                    # block of query chunk ni (interleaved into later rounds)
                    tb, half = c // 2, c % 2
                    b, r = tb // (512 // 128), tb % (512 // 128)
                    r0 = ni * 512 + r * 128
                    po = popp.tile([128, 512], F32, tag="po", name=f"po_{ni}_{c}")
                    nc.tensor.matmul(
                        po, av_n[b][:, r0:r0 + 128],
                        wo_full[:, half * 512:(half + 1) * 512],
                        start=True, stop=True,
                    )
                    os_ = osp.tile([128, 512], BF16, tag="os", name=f"os_{ni}_{c}")
                    nc.vector.tensor_copy(os_, po)
                    nc.sync.dma_start(
                        out=out[b * n_sz + r0:b * n_sz + r0 + 128,
                                half * 512:(half + 1) * 512],
                        in_=os_)

                r = 0  # global round counter (one round = one batch's pair)
                for ni in range(nic):
                    pav = {b: pavp.tile([128, 512], F32, tag=f"pav{b}",
                                        name=f"pav_{ni}_{b}") for b in range(b_sz)}
                    zt = pzp.tile([128, 512], F32, tag="zt", name=f"zt_{ni}")
                    prev = None
                    # hoist the whole chunk's bias DMAs: they sit in the sync
                    # queue AHEAD of any blocking output DMA, so the bias
                    # prefetch never starves the multiply stage
                    bts = []
                    for nj in range(njb):
                        bt2 = biasp.tile([128, HL, 512], BF16, tag="bt2",
                                         name=f"bt_{ni}_{nj}")
                        nc.sync.dma_start(
                            out=bt2,
                            in_=biasT[:, nj * 128:(nj + 1) * 128,
                                      ni * 512:(ni + 1) * 512]
                            .rearrange("h p q -> p h q"),
                        )
                        bts.append(bt2)
                    for nj in range(njb):
                        bt2 = bts[nj]
                        for b in range(b_sz):
                            sbuf = psim[r % 2]
                            aet, aut = ae[r % 3], au[r % 3]
                            # sims: the two heads go to different row groups
                            # AND different PSUM banks -> concurrent
                            for h in range(HL):
                                nc.tensor.matmul(
                                    sbuf[:, h * 512:(h + 1) * 512],
                                    kT[h * DH:(h + 1) * DH,
                                       b * n_sz + nj * 128:b * n_sz + (nj + 1) * 128],
                                    qT[h * DH:(h + 1) * DH,
                                       b * n_sz + ni * 512:b * n_sz + (ni + 1) * 512],
                                    start=True, stop=True,
                                )
                            # one exp spanning both heads' PSUM banks
                            nc.scalar.activation(out=aet, in_=sbuf, func=AF.Exp)
                            # bias multiply: ONE DVE op -> all four AV/Z
                            # matmuls become ready together and pack
                            nc.vector.tensor_mul(aut, aet, bt2)
                            # AV/Z for the PREVIOUS round (software pipeline)
                            if prev is not None:
                                av_z(prev[0], prev[1], au[prev[2] % 3], pav, zt)
                            prev = (b, nj, r)
                            r += 1
                        # interleave the previous query-chunk's output proj
                        if ni > 0:
                            po_chunk(ni - 1, nj)
                    av_z(prev[0], prev[1], au[prev[2] % 3], pav, zt)
                    # ---- ni boundary: evacuate pav, reciprocal Z, roundtrip ----
                    for b in range(b_sz):
                        nc.vector.tensor_copy(
                            av_u[b][:, ni * 512:(ni + 1) * 512], pav[b])
                    zrf = zc.tile([128, 512], F32, tag="zrf", name=f"zrf_{ni}")
                    nc.vector.reciprocal_approx_fast(zrf, zt)
                    zr = zc.tile([128, 512], BF16, tag="zr", name=f"zr_{ni}")
                    with nc.allow_low_precision(reason="1/Z bf16; ~4e-3 ok at 2e-2 gate"):
                        nc.vector.tensor_copy(zr, zrf)
                    for b in range(b_sz):
                        for h in range(HL):
                            s = b * HL + h
                            nc.gpsimd.dma_start(
                                out=zdram[b, h, 0, ni * 512:(ni + 1) * 512],
                                in_=zr[s * 32:s * 32 + 1, :])
                    for b in range(b_sz):
                        zb = zc.tile([128, 512], BF16, tag="zbb", name=f"zbb_{ni}_{b}")
                        for h in range(HL):
                            nc.gpsimd.dma_start(
                                out=zb[h * DH:(h + 1) * DH, :],
                                in_=zdram[b, h, :, ni * 512:(ni + 1) * 512]
                                .to_broadcast([DH, 512]))
                        with nc.allow_low_precision(reason="bf16 attention weights"):
                            nc.vector.tensor_mul(
                                av_n[b][:, ni * 512:(ni + 1) * 512],
                                av_u[b][:, ni * 512:(ni + 1) * 512],
                                zb)
            # ---- epilogue: last query-chunk's output projection ----
            with (
                tc.tile_pool(name="pout", bufs=4, space="PSUM") as pout,
                tc.tile_pool(name="ost2", bufs=6) as ost2,
            ):
                ni = nic - 1
                for c in range(2 * (512 // 128) * b_sz):
                    tb, half = c // 2, c % 2
                    b, rr = tb // (512 // 128), tb % (512 // 128)
                    r0 = ni * 512 + rr * 128
                    po = pout.tile([128, 512], F32, tag="po", name=f"poe_{c}")
                    nc.tensor.matmul(
                        po, av_n[b][:, r0:r0 + 128],
                        wo_full[:, half * 512:(half + 1) * 512],
                        start=True, stop=True,
                    )
                    os_ = ost2.tile([128, 512], BF16, tag="os", name=f"ose_{c}")
                    if c % 2 == 0:
                        nc.vector.tensor_copy(os_, po)
                    else:
                        nc.scalar.activation(out=os_, in_=po, func=AF.Copy)
                    nc.sync.dma_start(
                        out=out[b * n_sz + r0:b * n_sz + r0 + 128,
                                half * 512:(half + 1) * 512],
                        in_=os_)
    nc.compile()
    return nc


_NC_CACHE = {}


def _get_nc(key, **kw):
    if key not in _NC_CACHE:
        _NC_CACHE[key] = build(**kw)
    return _NC_CACHE[key]


def make_in_maps(x, rel_pos_bias, g, Wq, Wkv, Wo):
    b_sz, n_sz, dim = x.shape
    inner = Wq.shape[0]
    x = np.asarray(x, np.float32)
    xTh = np.ascontiguousarray(x.transpose(0, 2, 1)).astype(BF)  # [B, DIM, N]
    gv = np.asarray(g, np.float32).reshape(1, dim)
    Wq = np.asarray(Wq, np.float32) * gv
    Wkv = np.asarray(Wkv, np.float32) * gv
    scale = DH ** -0.5
    in_maps = []
    for c in range(NCORES):
        rs, re = c * IL, (c + 1) * IL
        wq_c = np.ascontiguousarray((Wq[rs:re, :] * scale).T).astype(BF)
        wk_c = np.ascontiguousarray(Wkv[rs:re, :].T).astype(BF)
        wv_c = np.ascontiguousarray(Wkv[inner + rs:inner + re, :].T).astype(BF)
        wo_c = np.ascontiguousarray(np.asarray(Wo)[:, rs:re].T).astype(BF)
        bias_c = np.exp(np.ascontiguousarray(
            np.asarray(rel_pos_bias)[0, c * HL:(c + 1) * HL].transpose(0, 2, 1)
        )).astype(BF)
        in_maps.append({
            "xT": xTh, "wqT": wq_c, "wkT": wk_c, "wvT": wv_c,
            "woT": wo_c, "biasT": bias_c,
        })
    return in_maps


def kernel(x, rel_pos_bias, g, Wq, Wkv, Wo):
    b_sz, n_sz, dim = x.shape
    nc = _get_nc((b_sz, n_sz, dim), b_sz=b_sz, n_sz=n_sz, dim=dim)
    in_maps = make_in_maps(x, rel_pos_bias, g, Wq, Wkv, Wo)
    res = run_bass_kernel_spmd(nc, in_maps, core_ids=list(range(NCORES)))
    acc = np.zeros((b_sz * n_sz, dim), np.float32)
    for r in res.results:
        acc += np.asarray(r["out"]).astype(np.float32)
    return np.ascontiguousarray(acc.reshape(b_sz, n_sz, dim))


# revision 18
# speedup vs baseline: 1.2675x; 1.0340x over previous
"""Trainium2 Bass kernel: multi-head attention with sequence-axis layernorm
and relative position bias, sharded 8-way over heads (2 heads/core).

v2 layout strategy (per core):
  - LN over sequence axis in [d_partition, n_free] layout; stats on DVE
    (bn_stats/bn_aggr), apply on DVE via fused tensor_scalar (x*scl + nshf)
    in bf16 (4x mode); g folded into Wq/Wkv on the host.
  - qT/kT [inner_local=128, b*n] via const-weight matmuls (K=128, Nf=512).
  - v natural per (b, nj): va_full[b,nj] [128 tokens, 128 inner] bf16.
  - attention rounds (ni, nj): all 4 streams (b x h) share ONE persistent
    4-bank PSUM tile [128, 2048] f32, cols [b0h0|b0h1|b1h0|b1h1]; the two
    sims of a batch are row-tiled (K=64 at row groups 0/64) into DIFFERENT
    banks and run concurrently on the PE.
  - exp: ONE ScalarE activation per batch-pair [128, 1024] spanning 2 PSUM
    banks (amortizes the ~352-cycle ACT instruction overhead).
  - bias folded multiplicatively: host precomputes exp(biasT) bf16; DVE
    multiplies (2x mode) into au.
  - AV: col-tiled pairs — h0 -> pav[b][0:64], h1 -> pav[b][64:128] (M=64,
    col groups disjoint -> concurrent), accumulated over nj. This yields a
    head-STACKED av [128, qi] enabling a K=128 output projection.
  - Z: separate [128,512] PSUM bank; 4 concurrent M=1 col-tiled matmuls
    with ones-weights at partitions {0,32,64,96}, accumulated over nj.
  - softmax denominator: reciprocal on DVE at ni boundary, DRAM roundtrip
    to broadcast 1/Z rows across 64 partitions; normalization deferred to
    the out-projection phase (off the attention critical path).
  - out-proj: stacked K=128 matmuls (lhsT = av_n [128, tok]), PSUM->SBUF
    copies alternating DVE/ACT, bf16 partial output summed on host in f32.
"""

import numpy as np
import ml_dtypes

import concourse.bass as bass
from concourse import bacc
import concourse.mybir as mybir
import concourse.tile as tile
from concourse.bass_utils import run_bass_kernel_spmd

F32 = mybir.dt.float32
BF16 = mybir.dt.bfloat16
BF = ml_dtypes.bfloat16
AF = mybir.ActivationFunctionType
ALU = mybir.AluOpType

# full-size problem constants
B, N, DIM = 2, 2048, 1024
HEADS, DH = 16, 64
NCORES = 8
HL = HEADS // NCORES          # heads per core = 2
IL = HL * DH                  # local inner = 128
INNER = HEADS * DH            # 1024


def build(b_sz=B, n_sz=N, dim=DIM, eps=1e-5):
    """Build the per-core Bass graph (SPMD across 8 cores)."""
    nd = dim // 128               # d tiles
    nch = (b_sz * n_sz) // 512    # 512-col chunks of flattened b*n
    njb = n_sz // 128             # key tiles per batch
    nic = n_sz // 512             # query chunks per batch
    bn = b_sz * n_sz
    nsub = n_sz // 512            # bn_stats subgroups

    nc = bacc.Bacc(None, target_bir_lowering=False)
    xT = nc.declare_dram_parameter("xT", [b_sz, dim, n_sz], BF16, isOutput=False)
    wqT = nc.declare_dram_parameter("wqT", [dim, IL], BF16, isOutput=False)
    wkT = nc.declare_dram_parameter("wkT", [dim, IL], BF16, isOutput=False)
    wvT = nc.declare_dram_parameter("wvT", [dim, IL], BF16, isOutput=False)
    woT = nc.declare_dram_parameter("woT", [IL, dim], BF16, isOutput=False)
    biasT = nc.declare_dram_parameter("biasT", [HL, n_sz, n_sz], BF16, isOutput=False)  # exp(bias.T)
    out = nc.declare_dram_parameter("out", [bn, dim], BF16, isOutput=True)
    zdram = nc.dram_tensor("zscratch", [b_sz, HL, 1, n_sz], BF16)

    with tile.TileContext(nc) as tc:
        with (
            tc.tile_pool(name="consts", bufs=1) as consts,
            tc.tile_pool(name="persist", bufs=1) as persist,
        ):
            # ---- load weights ----
            wq_s, wk_s, wv_s = [], [], []
            for dt in range(nd):
                for lst, src, nm in ((wq_s, wqT, "wq"), (wk_s, wkT, "wk"), (wv_s, wvT, "wv")):
                    t = consts.tile([128, IL], BF16, tag=f"{nm}{dt}")
                    nc.sync.dma_start(out=t, in_=src[dt * 128:(dt + 1) * 128, :])
                    lst.append(t)
            wo_full = consts.tile([IL, dim], BF16, tag="wo")
            nc.sync.dma_start(out=wo_full, in_=woT[:, :])
            ones = consts.tile([128, 1], BF16, tag="ones")
            nc.vector.memset(ones, 1.0)

            xn = {}
            qT = persist.tile([IL, bn], BF16, tag="qT")
            kT = persist.tile([IL, bn], BF16, tag="kT")
            va = {}   # (b, nj) -> [128 tokens, 128 inner] bf16

            # ---- Phase 1: layernorm over sequence axis ----
            # mean/var via DVE bn_stats for half the tiles, via ScalarE
            # Square/Identity+accum_out for the other half (the two engines
            # run the stats concurrently; DVE was the pre-phase bottleneck)
            inv_n = 1.0 / n_sz
            with (
                tc.tile_pool(name="xload", bufs=4) as xload,
                tc.tile_pool(name="lns", bufs=8) as lns,
                tc.tile_pool(name="lnscr", bufs=2) as lnscr,
            ):
                for b in range(b_sz):
                    for dt in range(nd):
                        xt = xload.tile([128, n_sz], BF16, tag="xt",
                                        name=f"xt_{b}_{dt}")
                        nc.sync.dma_start(out=xt, in_=xT[b, dt * 128:(dt + 1) * 128, :])
                        mv = lns.tile([128, 2], F32, tag="mv", name=f"mv_{b}_{dt}")
                        if (b * nd + dt) % 2 == 0:
                            stats = lns.tile([128, nsub, 6], F32, tag="stats",
                                             name=f"st_{b}_{dt}")
                            for s in range(nsub):
                                nc.vector.bn_stats(out=stats[:, s, :],
                                                   in_=xt[:, s * 512:(s + 1) * 512])
                            nc.vector.bn_aggr(out=mv, in_=stats)
                        else:
                            scr = lnscr.tile([128, n_sz], BF16, tag="scr",
                                             name=f"scr_{b}_{dt}")
                            sums = lns.tile([128, 2], F32, tag="sums",
                                            name=f"sums_{b}_{dt}")
                            nc.scalar.activation(out=scr, in_=xt, func=AF.Identity,
                                                 accum_out=sums[:, 0:1])
                            nc.scalar.activation(out=scr, in_=xt, func=AF.Square,
                                                 accum_out=sums[:, 1:2])
                            # mean = sum/n ; var = sumsq/n - mean^2
                            nc.vector.tensor_scalar_mul(mv[:, 0:1], sums[:, 0:1], inv_n)
                            msq = lns.tile([128, 1], F32, tag="msq",
                                           name=f"msq_{b}_{dt}")
                            nc.vector.tensor_mul(msq, mv[:, 0:1], mv[:, 0:1])
                            with nc.allow_low_precision(reason="var f32"):
                                nc.vector.tensor_scalar(
                                    mv[:, 1:2], sums[:, 1:2], inv_n, msq,
                                    ALU.mult, ALU.subtract)
                        vmax = lns.tile([128, 1], F32, tag="vmax", name=f"vm_{b}_{dt}")
                        nc.vector.tensor_scalar_max(vmax, mv[:, 1:2], eps)
                        sq = lns.tile([128, 1], F32, tag="sq", name=f"sq_{b}_{dt}")
                        nc.scalar.activation(out=sq, in_=vmax, func=AF.Sqrt)
                        scl = lns.tile([128, 1], F32, tag="scl", name=f"scl_{b}_{dt}")
                        nc.vector.reciprocal(scl, sq)
                        nshf = lns.tile([128, 1], F32, tag="nshf", name=f"ns_{b}_{dt}")
                        with nc.allow_low_precision(reason="mean*scl in f32; fine"):
                            nc.vector.tensor_scalar(
                                nshf, mv[:, 0:1], scl, -1.0, ALU.mult, ALU.mult)
                        xnt = persist.tile([128, n_sz], BF16, tag=f"xn_{b}_{dt}")
                        with nc.allow_low_precision(reason="bf16 LN apply; ~4e-3 ok"):
                            nc.vector.tensor_scalar(
                                xnt, xt, scl, nshf, ALU.mult, ALU.add)
                        xn[b, dt] = xnt

            # ---- Phase 2a: k projection (all chunks) + q for chunk ni=0;
            # q for ni>=1 is deferred into the attention loop (through the
            # spare PSUM slot) to shorten the pre-phase PE chain ----
            with tc.tile_pool(name="pproj", bufs=4, space="PSUM") as pproj:
                for ch in range(nch):
                    b = (ch * 512) // n_sz
                    col0 = (ch * 512) % n_sz
                    ps = pproj.tile([IL, 512], F32, tag="ps", name=f"psk_{ch}")
                    for dt in range(nd):
                        nc.tensor.matmul(
                            ps, wk_s[dt], xn[b, dt][:, col0:col0 + 512],
                            start=(dt == 0), stop=(dt == nd - 1),
                        )
                    nc.scalar.activation(out=kT[:, ch * 512:(ch + 1) * 512],
                                         in_=ps, func=AF.Copy)
                for b in range(b_sz):
                    ps = pproj.tile([IL, 512], F32, tag="ps", name=f"psq0_{b}")
                    for dt in range(nd):
                        nc.tensor.matmul(
                            ps, wq_s[dt], xn[b, dt][:, 0:512],
                            start=(dt == 0), stop=(dt == nd - 1),
                        )
                    nc.scalar.activation(out=qT[:, b * n_sz:b * n_sz + 512],
                                         in_=ps, func=AF.Copy)

            # ---- Phase 2b: v natural [tokens, inner] ----
            with tc.tile_pool(name="pv", bufs=4, space="PSUM") as pv:
                for b in range(b_sz):
                    for nj in range(njb):
                        psv = pv.tile([128, IL], F32, tag="psv", name=f"psv_{b}_{nj}")
                        for dt in range(nd):
                            nc.tensor.matmul(
                                psv, xn[b, dt][:, nj * 128:(nj + 1) * 128], wv_s[dt],
                                start=(dt == 0), stop=(dt == nd - 1),
                            )
                        t = persist.tile([128, IL], BF16, tag=f"va_{b}_{nj}")
                        nc.scalar.activation(out=t, in_=psv, func=AF.Copy)
                        va[b, nj] = t

            # ---- Phase 3: attention ----
            # stream -> psim column range: [b0h0 | b0h1 | b1h0 | b1h1]
            # (each 512 f32 = exactly one PSUM bank; b-pairs adjacent so one
            # 1024-wide exp covers both heads of a batch)
            av_u = {b: persist.tile([128, n_sz], BF16, tag=f"avu_{b}",
                                    name=f"avu_{b}")
                    for b in range(b_sz)}
            av_n = {b: persist.tile([128, n_sz], BF16, tag=f"avn_{b}",
                                    name=f"avn_{b}")
                    for b in range(b_sz)}
            zbb = {}  # (b, ni) -> [128, 512] bf16 stacked 1/Z broadcast
            with (
                tc.tile_pool(name="psim", bufs=1, space="PSUM") as psimp,
                tc.tile_pool(name="pav", bufs=1, space="PSUM") as pavp,
                tc.tile_pool(name="pz", bufs=1, space="PSUM") as pzp,
                tc.tile_pool(name="pop", bufs=1, space="PSUM") as popp,
                tc.tile_pool(name="biasp", bufs=16) as biasp,
                tc.tile_pool(name="aep", bufs=1) as aep,
                tc.tile_pool(name="osp", bufs=4) as osp,
                tc.tile_pool(name="zc", bufs=4) as zc,
            ):
                # two 2-bank sim buffers: exp(r) reads one while the next
                # round's sims fill the other (the exp pipeline never stalls)
                psim = [psimp.tile([128, HL * 512], F32, tag=f"psim{i}",
                                   name=f"psim{i}") for i in range(2)]
                ae = [aep.tile([128, HL * 512], BF16, tag=f"ae{i}", name=f"ae{i}")
                      for i in range(3)]
                au = [aep.tile([128, HL * 512], BF16, tag=f"au{i}", name=f"au{i}")
                      for i in range(3)]

                def av_z(b, nj, aut, pav, zt):
                    # AV: col-tiled h-pair (concurrent), head-stacked output;
                    # Z: 2 concurrent M=1 col-tiled matmuls
                    for h in range(HL):
                        nc.tensor.matmul(
                            pav[b][h * DH:(h + 1) * DH, :],
                            va[b, nj][:, h * DH:(h + 1) * DH],
                            aut[:, h * 512:(h + 1) * 512],
                            start=(nj == 0), stop=(nj == njb - 1),
                            tile_position=(0, h * DH),
                        )
                    for h in range(HL):
                        s = b * HL + h
                        nc.tensor.matmul(
                            zt[s * 32:s * 32 + 1, :],
                            ones,
                            aut[:, h * 512:(h + 1) * 512],
                            start=(nj == 0), stop=(nj == njb - 1),
                            tile_position=(0, s * 32),
                        )

                def po_chunk(ni, c):
                    # output projection for one 512-dim half of a 128-token
                    # block of query chunk ni (interleaved into later rounds)
                    tb, half = c // 2, c % 2
                    b, r = tb // (512 // 128), tb % (512 // 128)
                    r0 = ni * 512 + r * 128
                    po = popp.tile([128, 512], F32, tag="po", name=f"po_{ni}_{c}")
                    nc.tensor.matmul(
                        po, av_n[b][:, r0:r0 + 128],
                        wo_full[:, half * 512:(half + 1) * 512],
                        start=True, stop=True,
                    )
                    os_ = osp.tile([128, 512], BF16, tag="os", name=f"os_{ni}_{c}")
                    nc.vector.tensor_copy(os_, po)
                    nc.sync.dma_start(
                        out=out[b * n_sz + r0:b * n_sz + r0 + 128,
                                half * 512:(half + 1) * 512],
                        in_=os_)

                r = 0  # global round counter (one round = one batch's pair)
                for ni in range(nic):
                    pav = {b: pavp.tile([128, 512], F32, tag=f"pav{b}",
                                        name=f"pav_{ni}_{b}") for b in range(b_sz)}
                    zt = pzp.tile([128, 512], F32, tag="zt", name=f"zt_{ni}")
                    prev = None
                    # deferred q projection for the NEXT query chunk, using
                    # the shared spare PSUM slot (idle for po during ni=0)
                    if ni + 1 < nic:
                        for b in range(b_sz):
                            qps = popp.tile([128, 512], F32, tag="po",
                                            name=f"qps_{ni + 1}_{b}")
                            for dt in range(nd):
                                nc.tensor.matmul(
                                    qps, wq_s[dt],
                                    xn[b, dt][:, (ni + 1) * 512:(ni + 2) * 512],
                                    start=(dt == 0), stop=(dt == nd - 1),
                                )
                            c0 = b * n_sz + (ni + 1) * 512
                            nc.vector.tensor_copy(qT[:, c0:c0 + 512], qps)
                    # hoist the whole chunk's bias DMAs: they sit in the sync
                    # queue AHEAD of any blocking output DMA, so the bias
                    # prefetch never starves the multiply stage
                    bts = []
                    for nj in range(njb):
                        bt2 = biasp.tile([128, HL, 512], BF16, tag="bt2",
                                         name=f"bt_{ni}_{nj}")
                        nc.sync.dma_start(
                            out=bt2,
                            in_=biasT[:, nj * 128:(nj + 1) * 128,
                                      ni * 512:(ni + 1) * 512]
                            .rearrange("h p q -> p h q"),
                        )
                        bts.append(bt2)
                    for nj in range(njb):
                        bt2 = bts[nj]
                        for b in range(b_sz):
                            sbuf = psim[r % 2]
                            aet, aut = ae[r % 3], au[r % 3]
                            # sims: the two heads go to different row groups
                            # AND different PSUM banks -> concurrent
                            for h in range(HL):
                                nc.tensor.matmul(
                                    sbuf[:, h * 512:(h + 1) * 512],
                                    kT[h * DH:(h + 1) * DH,
                                       b * n_sz + nj * 128:b * n_sz + (nj + 1) * 128],
                                    qT[h * DH:(h + 1) * DH,
                                       b * n_sz + ni * 512:b * n_sz + (ni + 1) * 512],
                                    start=True, stop=True,
                                )
                            # one exp spanning both heads' PSUM banks
                            nc.scalar.activation(out=aet, in_=sbuf, func=AF.Exp)
                            # bias multiply: ONE DVE op -> all four AV/Z
                            # matmuls become ready together and pack
                            nc.vector.tensor_mul(aut, aet, bt2)
                            # AV/Z for the PREVIOUS round (software pipeline)
                            if prev is not None:
                                av_z(prev[0], prev[1], au[prev[2] % 3], pav, zt)
                            prev = (b, nj, r)
                            r += 1
                        # interleave the previous query-chunk's output proj
                        if ni > 0:
                            po_chunk(ni - 1, nj)
                    av_z(prev[0], prev[1], au[prev[2] % 3], pav, zt)
                    # ---- ni boundary: evacuate pav, reciprocal Z, roundtrip ----
                    for b in range(b_sz):
                        nc.vector.tensor_copy(
                            av_u[b][:, ni * 512:(ni + 1) * 512], pav[b])
                    zrf = zc.tile([128, 512], F32, tag="zrf", name=f"zrf_{ni}")
                    nc.vector.reciprocal_approx_fast(zrf, zt)
                    zr = zc.tile([128, 512], BF16, tag="zr", name=f"zr_{ni}")
                    with nc.allow_low_precision(reason="1/Z bf16; ~4e-3 ok at 2e-2 gate"):
                        nc.vector.tensor_copy(zr, zrf)
                    for b in range(b_sz):
                        for h in range(HL):
                            s = b * HL + h
                            nc.gpsimd.dma_start(
                                out=zdram[b, h, 0, ni * 512:(ni + 1) * 512],
                                in_=zr[s * 32:s * 32 + 1, :])
                    for b in range(b_sz):
                        zb = zc.tile([128, 512], BF16, tag="zbb", name=f"zbb_{ni}_{b}")
                        for h in range(HL):
                            nc.gpsimd.dma_start(
                                out=zb[h * DH:(h + 1) * DH, :],
                                in_=zdram[b, h, :, ni * 512:(ni + 1) * 512]
                                .to_broadcast([DH, 512]))
                        with nc.allow_low_precision(reason="bf16 attention weights"):
                            nc.vector.tensor_mul(
                                av_n[b][:, ni * 512:(ni + 1) * 512],
                                av_u[b][:, ni * 512:(ni + 1) * 512],
                                zb)
            # ---- epilogue: last query-chunk's output projection ----
            with (
                tc.tile_pool(name="pout", bufs=4, space="PSUM") as pout,
                tc.tile_pool(name="ost2", bufs=6) as ost2,
            ):
                ni = nic - 1
                for c in range(2 * (512 // 128) * b_sz):
                    tb, half = c // 2, c % 2
                    b, rr = tb // (512 // 128), tb % (512 // 128)
                    r0 = ni * 512 + rr * 128
                    po = pout.tile([128, 512], F32, tag="po", name=f"poe_{c}")
                    nc.tensor.matmul(
                        po, av_n[b][:, r0:r0 + 128],
                        wo_full[:, half * 512:(half + 1) * 512],
                        start=True, stop=True,
                    )
                    os_ = ost2.tile([128, 512], BF16, tag="os", name=f"ose_{c}")
                    if c % 2 == 0:
                        nc.vector.tensor_copy(os_, po)
                    else:
                        nc.scalar.activation(out=os_, in_=po, func=AF.Copy)
                    nc.sync.dma_start(
                        out=out[b * n_sz + r0:b * n_sz + r0 + 128,
                                half * 512:(half + 1) * 512],
                        in_=os_)
    nc.compile()
    return nc


_NC_CACHE = {}


def _get_nc(key, **kw):
    if key not in _NC_CACHE:
        _NC_CACHE[key] = build(**kw)
    return _NC_CACHE[key]


def make_in_maps(x, rel_pos_bias, g, Wq, Wkv, Wo):
    b_sz, n_sz, dim = x.shape
    inner = Wq.shape[0]
    x = np.asarray(x, np.float32)
    xTh = np.ascontiguousarray(x.transpose(0, 2, 1)).astype(BF)  # [B, DIM, N]
    gv = np.asarray(g, np.float32).reshape(1, dim)
    Wq = np.asarray(Wq, np.float32) * gv
    Wkv = np.asarray(Wkv, np.float32) * gv
    scale = DH ** -0.5
    in_maps = []
    for c in range(NCORES):
        rs, re = c * IL, (c + 1) * IL
        wq_c = np.ascontiguousarray((Wq[rs:re, :] * scale).T).astype(BF)
        wk_c = np.ascontiguousarray(Wkv[rs:re, :].T).astype(BF)
        wv_c = np.ascontiguousarray(Wkv[inner + rs:inner + re, :].T).astype(BF)
        wo_c = np.ascontiguousarray(np.asarray(Wo)[:, rs:re].T).astype(BF)
        bias_c = np.exp(np.ascontiguousarray(
            np.asarray(rel_pos_bias)[0, c * HL:(c + 1) * HL].transpose(0, 2, 1)
        )).astype(BF)
        in_maps.append({
            "xT": xTh, "wqT": wq_c, "wkT": wk_c, "wvT": wv_c,
            "woT": wo_c, "biasT": bias_c,
        })
    return in_maps


def kernel(x, rel_pos_bias, g, Wq, Wkv, Wo):
    b_sz, n_sz, dim = x.shape
    nc = _get_nc((b_sz, n_sz, dim), b_sz=b_sz, n_sz=n_sz, dim=dim)
    in_maps = make_in_maps(x, rel_pos_bias, g, Wq, Wkv, Wo)
    res = run_bass_kernel_spmd(nc, in_maps, core_ids=list(range(NCORES)))
    acc = np.zeros((b_sz * n_sz, dim), np.float32)
    for r in res.results:
        acc += np.asarray(r["out"]).astype(np.float32)
    return np.ascontiguousarray(acc.reshape(b_sz, n_sz, dim))


# revision 19
# speedup vs baseline: 1.2742x; 1.0053x over previous
"""Trainium2 Bass kernel: multi-head attention with sequence-axis layernorm
and relative position bias, sharded 8-way over heads (2 heads/core).

v2 layout strategy (per core):
  - LN over sequence axis in [d_partition, n_free] layout; stats on DVE
    (bn_stats/bn_aggr), apply on DVE via fused tensor_scalar (x*scl + nshf)
    in bf16 (4x mode); g folded into Wq/Wkv on the host.
  - qT/kT [inner_local=128, b*n] via const-weight matmuls (K=128, Nf=512).
  - v natural per (b, nj): va_full[b,nj] [128 tokens, 128 inner] bf16.
  - attention rounds (ni, nj): all 4 streams (b x h) share ONE persistent
    4-bank PSUM tile [128, 2048] f32, cols [b0h0|b0h1|b1h0|b1h1]; the two
    sims of a batch are row-tiled (K=64 at row groups 0/64) into DIFFERENT
    banks and run concurrently on the PE.
  - exp: ONE ScalarE activation per batch-pair [128, 1024] spanning 2 PSUM
    banks (amortizes the ~352-cycle ACT instruction overhead).
  - bias folded multiplicatively: host precomputes exp(biasT) bf16; DVE
    multiplies (2x mode) into au.
  - AV: col-tiled pairs — h0 -> pav[b][0:64], h1 -> pav[b][64:128] (M=64,
    col groups disjoint -> concurrent), accumulated over nj. This yields a
    head-STACKED av [128, qi] enabling a K=128 output projection.
  - Z: separate [128,512] PSUM bank; 4 concurrent M=1 col-tiled matmuls
    with ones-weights at partitions {0,32,64,96}, accumulated over nj.
  - softmax denominator: reciprocal on DVE at ni boundary, DRAM roundtrip
    to broadcast 1/Z rows across 64 partitions; normalization deferred to
    the out-projection phase (off the attention critical path).
  - out-proj: stacked K=128 matmuls (lhsT = av_n [128, tok]), PSUM->SBUF
    copies alternating DVE/ACT, bf16 partial output summed on host in f32.
"""

import numpy as np
import ml_dtypes

import concourse.bass as bass
from concourse import bacc
import concourse.mybir as mybir
import concourse.tile as tile
from concourse.bass_utils import run_bass_kernel_spmd

F32 = mybir.dt.float32
BF16 = mybir.dt.bfloat16
BF = ml_dtypes.bfloat16
AF = mybir.ActivationFunctionType
ALU = mybir.AluOpType

# full-size problem constants
B, N, DIM = 2, 2048, 1024
HEADS, DH = 16, 64
NCORES = 8
HL = HEADS // NCORES          # heads per core = 2
IL = HL * DH                  # local inner = 128
INNER = HEADS * DH            # 1024


def build(b_sz=B, n_sz=N, dim=DIM, eps=1e-5):
    """Build the per-core Bass graph (SPMD across 8 cores)."""
    nd = dim // 128               # d tiles
    nch = (b_sz * n_sz) // 512    # 512-col chunks of flattened b*n
    njb = n_sz // 128             # key tiles per batch
    nic = n_sz // 512             # query chunks per batch
    bn = b_sz * n_sz
    nsub = n_sz // 512            # bn_stats subgroups

    nc = bacc.Bacc(None, target_bir_lowering=False)
    xT = nc.declare_dram_parameter("xT", [b_sz, dim, n_sz], BF16, isOutput=False)
    wqT = nc.declare_dram_parameter("wqT", [dim, IL], BF16, isOutput=False)
    wkT = nc.declare_dram_parameter("wkT", [dim, IL], BF16, isOutput=False)
    wvT = nc.declare_dram_parameter("wvT", [dim, IL], BF16, isOutput=False)
    woT = nc.declare_dram_parameter("woT", [IL, dim], BF16, isOutput=False)
    biasT = nc.declare_dram_parameter("biasT", [HL, n_sz, n_sz], BF16, isOutput=False)  # exp(bias.T)
    out = nc.declare_dram_parameter("out", [bn, dim], BF16, isOutput=True)
    zdram = nc.dram_tensor("zscratch", [b_sz, HL, 1, n_sz], BF16)

    with tile.TileContext(nc) as tc:
        with (
            tc.tile_pool(name="consts", bufs=1) as consts,
            tc.tile_pool(name="persist", bufs=1) as persist,
        ):
            # ---- load weights ----
            wq_s, wk_s, wv_s = [], [], []
            for dt in range(nd):
                for lst, src, nm in ((wq_s, wqT, "wq"), (wk_s, wkT, "wk"), (wv_s, wvT, "wv")):
                    t = consts.tile([128, IL], BF16, tag=f"{nm}{dt}")
                    nc.sync.dma_start(out=t, in_=src[dt * 128:(dt + 1) * 128, :])
                    lst.append(t)
            wo_full = consts.tile([IL, dim], BF16, tag="wo")
            nc.sync.dma_start(out=wo_full, in_=woT[:, :])
            ones = consts.tile([128, 1], BF16, tag="ones")
            nc.vector.memset(ones, 1.0)

            xn = {}
            qT = persist.tile([IL, bn], BF16, tag="qT")
            kT = persist.tile([IL, bn], BF16, tag="kT")
            va = {}   # (b, nj) -> [128 tokens, 128 inner] bf16

            # ---- Phase 1: layernorm over sequence axis ----
            # mean/var via DVE bn_stats for half the tiles, via ScalarE
            # Square/Identity+accum_out for the other half (the two engines
            # run the stats concurrently; DVE was the pre-phase bottleneck)
            inv_n = 1.0 / n_sz
            with (
                tc.tile_pool(name="xload", bufs=6) as xload,
                tc.tile_pool(name="lns", bufs=8) as lns,
                tc.tile_pool(name="lnscr", bufs=2) as lnscr,
            ):
                for b in range(b_sz):
                    for dt in range(nd):
                        xt = xload.tile([128, n_sz], BF16, tag="xt",
                                        name=f"xt_{b}_{dt}")
                        nc.sync.dma_start(out=xt, in_=xT[b, dt * 128:(dt + 1) * 128, :])
                        mv = lns.tile([128, 2], F32, tag="mv", name=f"mv_{b}_{dt}")
                        if (b * nd + dt) % 3 != 2:
                            stats = lns.tile([128, nsub, 6], F32, tag="stats",
                                             name=f"st_{b}_{dt}")
                            for s in range(nsub):
                                nc.vector.bn_stats(out=stats[:, s, :],
                                                   in_=xt[:, s * 512:(s + 1) * 512])
                            nc.vector.bn_aggr(out=mv, in_=stats)
                        else:
                            scr = lnscr.tile([128, n_sz], BF16, tag="scr",
                                             name=f"scr_{b}_{dt}")
                            sums = lns.tile([128, 2], F32, tag="sums",
                                            name=f"sums_{b}_{dt}")
                            nc.scalar.activation(out=scr, in_=xt, func=AF.Identity,
                                                 accum_out=sums[:, 0:1])
                            nc.scalar.activation(out=scr, in_=xt, func=AF.Square,
                                                 accum_out=sums[:, 1:2])
                            # mean = sum/n ; var = sumsq/n - mean^2
                            nc.vector.tensor_scalar_mul(mv[:, 0:1], sums[:, 0:1], inv_n)
                            msq = lns.tile([128, 1], F32, tag="msq",
                                           name=f"msq_{b}_{dt}")
                            nc.vector.tensor_mul(msq, mv[:, 0:1], mv[:, 0:1])
                            with nc.allow_low_precision(reason="var f32"):
                                nc.vector.tensor_scalar(
                                    mv[:, 1:2], sums[:, 1:2], inv_n, msq,
                                    ALU.mult, ALU.subtract)
                        vmax = lns.tile([128, 1], F32, tag="vmax", name=f"vm_{b}_{dt}")
                        nc.vector.tensor_scalar_max(vmax, mv[:, 1:2], eps)
                        sq = lns.tile([128, 1], F32, tag="sq", name=f"sq_{b}_{dt}")
                        nc.scalar.activation(out=sq, in_=vmax, func=AF.Sqrt)
                        scl = lns.tile([128, 1], F32, tag="scl", name=f"scl_{b}_{dt}")
                        nc.vector.reciprocal(scl, sq)
                        nshf = lns.tile([128, 1], F32, tag="nshf", name=f"ns_{b}_{dt}")
                        with nc.allow_low_precision(reason="mean*scl in f32; fine"):
                            nc.vector.tensor_scalar(
                                nshf, mv[:, 0:1], scl, -1.0, ALU.mult, ALU.mult)
                        xnt = persist.tile([128, n_sz], BF16, tag=f"xn_{b}_{dt}")
                        with nc.allow_low_precision(reason="bf16 LN apply; ~4e-3 ok"):
                            nc.vector.tensor_scalar(
                                xnt, xt, scl, nshf, ALU.mult, ALU.add)
                        xn[b, dt] = xnt

            # ---- Phase 2a: k projection (all chunks) + q for chunk ni=0;
            # q for ni>=1 is deferred into the attention loop (through the
            # spare PSUM slot) to shorten the pre-phase PE chain ----
            with tc.tile_pool(name="pproj", bufs=4, space="PSUM") as pproj:
                for ch in range(nch):
                    b = (ch * 512) // n_sz
                    col0 = (ch * 512) % n_sz
                    ps = pproj.tile([IL, 512], F32, tag="ps", name=f"psk_{ch}")
                    for dt in range(nd):
                        nc.tensor.matmul(
                            ps, wk_s[dt], xn[b, dt][:, col0:col0 + 512],
                            start=(dt == 0), stop=(dt == nd - 1),
                        )
                    nc.scalar.activation(out=kT[:, ch * 512:(ch + 1) * 512],
                                         in_=ps, func=AF.Copy)
                for b in range(b_sz):
                    ps = pproj.tile([IL, 512], F32, tag="ps", name=f"psq0_{b}")
                    for dt in range(nd):
                        nc.tensor.matmul(
                            ps, wq_s[dt], xn[b, dt][:, 0:512],
                            start=(dt == 0), stop=(dt == nd - 1),
                        )
                    nc.scalar.activation(out=qT[:, b * n_sz:b * n_sz + 512],
                                         in_=ps, func=AF.Copy)

            # ---- Phase 2b: v natural [tokens, inner] ----
            with tc.tile_pool(name="pv", bufs=4, space="PSUM") as pv:
                for b in range(b_sz):
                    for nj in range(njb):
                        psv = pv.tile([128, IL], F32, tag="psv", name=f"psv_{b}_{nj}")
                        for dt in range(nd):
                            nc.tensor.matmul(
                                psv, xn[b, dt][:, nj * 128:(nj + 1) * 128], wv_s[dt],
                                start=(dt == 0), stop=(dt == nd - 1),
                            )
                        t = persist.tile([128, IL], BF16, tag=f"va_{b}_{nj}")
                        nc.scalar.activation(out=t, in_=psv, func=AF.Copy)
                        va[b, nj] = t

            # ---- Phase 3: attention ----
            # stream -> psim column range: [b0h0 | b0h1 | b1h0 | b1h1]
            # (each 512 f32 = exactly one PSUM bank; b-pairs adjacent so one
            # 1024-wide exp covers both heads of a batch)
            av_u = {b: persist.tile([128, n_sz], BF16, tag=f"avu_{b}",
                                    name=f"avu_{b}")
                    for b in range(b_sz)}
            av_n = {b: persist.tile([128, n_sz], BF16, tag=f"avn_{b}",
                                    name=f"avn_{b}")
                    for b in range(b_sz)}
            zbb = {}  # (b, ni) -> [128, 512] bf16 stacked 1/Z broadcast
            with (
                tc.tile_pool(name="psim", bufs=1, space="PSUM") as psimp,
                tc.tile_pool(name="pav", bufs=1, space="PSUM") as pavp,
                tc.tile_pool(name="pz", bufs=1, space="PSUM") as pzp,
                tc.tile_pool(name="pop", bufs=1, space="PSUM") as popp,
                tc.tile_pool(name="biasp", bufs=16) as biasp,
                tc.tile_pool(name="aep", bufs=1) as aep,
                tc.tile_pool(name="osp", bufs=6) as osp,
                tc.tile_pool(name="zc", bufs=4) as zc,
            ):
                # two 2-bank sim buffers: exp(r) reads one while the next
                # round's sims fill the other (the exp pipeline never stalls)
                psim = [psimp.tile([128, HL * 512], F32, tag=f"psim{i}",
                                   name=f"psim{i}") for i in range(2)]
                ae = [aep.tile([128, HL * 512], BF16, tag=f"ae{i}", name=f"ae{i}")
                      for i in range(3)]
                au = [aep.tile([128, HL * 512], BF16, tag=f"au{i}", name=f"au{i}")
                      for i in range(3)]

                def av_z(b, nj, aut, pav, zt):
                    # AV: col-tiled h-pair (concurrent), head-stacked output;
                    # Z: 2 concurrent M=1 col-tiled matmuls
                    for h in range(HL):
                        nc.tensor.matmul(
                            pav[b][h * DH:(h + 1) * DH, :],
                            va[b, nj][:, h * DH:(h + 1) * DH],
                            aut[:, h * 512:(h + 1) * 512],
                            start=(nj == 0), stop=(nj == njb - 1),
                            tile_position=(0, h * DH),
                        )
                    for h in range(HL):
                        s = b * HL + h
                        nc.tensor.matmul(
                            zt[s * 32:s * 32 + 1, :],
                            ones,
                            aut[:, h * 512:(h + 1) * 512],
                            start=(nj == 0), stop=(nj == njb - 1),
                            tile_position=(0, s * 32),
                        )

                def po_chunk(ni, c):
                    # output projection for one 512-dim half of a 128-token
                    # block of query chunk ni (interleaved into later rounds)
                    tb, half = c // 2, c % 2
                    b, r = tb // (512 // 128), tb % (512 // 128)
                    r0 = ni * 512 + r * 128
                    po = popp.tile([128, 512], F32, tag="po", name=f"po_{ni}_{c}")
                    nc.tensor.matmul(
                        po, av_n[b][:, r0:r0 + 128],
                        wo_full[:, half * 512:(half + 1) * 512],
                        start=True, stop=True,
                    )
                    os_ = osp.tile([128, 512], BF16, tag="os", name=f"os_{ni}_{c}")
                    nc.vector.tensor_copy(os_, po)
                    nc.sync.dma_start(
                        out=out[b * n_sz + r0:b * n_sz + r0 + 128,
                                half * 512:(half + 1) * 512],
                        in_=os_)

                r = 0  # global round counter (one round = one batch's pair)
                for ni in range(nic):
                    pav = {b: pavp.tile([128, 512], F32, tag=f"pav{b}",
                                        name=f"pav_{ni}_{b}") for b in range(b_sz)}
                    zt = pzp.tile([128, 512], F32, tag="zt", name=f"zt_{ni}")
                    prev = None
                    # deferred q projection for the NEXT query chunk, using
                    # the shared spare PSUM slot (idle for po during ni=0)
                    if ni + 1 < nic:
                        for b in range(b_sz):
                            qps = popp.tile([128, 512], F32, tag="po",
                                            name=f"qps_{ni + 1}_{b}")
                            for dt in range(nd):
                                nc.tensor.matmul(
                                    qps, wq_s[dt],
                                    xn[b, dt][:, (ni + 1) * 512:(ni + 2) * 512],
                                    start=(dt == 0), stop=(dt == nd - 1),
                                )
                            c0 = b * n_sz + (ni + 1) * 512
                            nc.vector.tensor_copy(qT[:, c0:c0 + 512], qps)
                    # hoist the whole chunk's bias DMAs: they sit in the sync
                    # queue AHEAD of any blocking output DMA, so the bias
                    # prefetch never starves the multiply stage
                    bts = []
                    for nj in range(njb):
                        bt2 = biasp.tile([128, HL, 512], BF16, tag="bt2",
                                         name=f"bt_{ni}_{nj}")
                        nc.sync.dma_start(
                            out=bt2,
                            in_=biasT[:, nj * 128:(nj + 1) * 128,
                                      ni * 512:(ni + 1) * 512]
                            .rearrange("h p q -> p h q"),
                        )
                        bts.append(bt2)
                    for nj in range(njb):
                        bt2 = bts[nj]
                        for b in range(b_sz):
                            sbuf = psim[r % 2]
                            aet, aut = ae[r % 3], au[r % 3]
                            # sims: the two heads go to different row groups
                            # AND different PSUM banks -> concurrent
                            for h in range(HL):
                                nc.tensor.matmul(
                                    sbuf[:, h * 512:(h + 1) * 512],
                                    kT[h * DH:(h + 1) * DH,
                                       b * n_sz + nj * 128:b * n_sz + (nj + 1) * 128],
                                    qT[h * DH:(h + 1) * DH,
                                       b * n_sz + ni * 512:b * n_sz + (ni + 1) * 512],
                                    start=True, stop=True,
                                )
                            # one exp spanning both heads' PSUM banks
                            nc.scalar.activation(out=aet, in_=sbuf, func=AF.Exp)
                            # bias multiply: ONE DVE op -> all four AV/Z
                            # matmuls become ready together and pack
                            nc.vector.tensor_mul(aut, aet, bt2)
                            # AV/Z for the PREVIOUS round (software pipeline)
                            if prev is not None:
                                av_z(prev[0], prev[1], au[prev[2] % 3], pav, zt)
                            prev = (b, nj, r)
                            r += 1
                        # interleave the previous query-chunk's output proj
                        if ni > 0:
                            po_chunk(ni - 1, nj)
                    av_z(prev[0], prev[1], au[prev[2] % 3], pav, zt)
                    # ---- ni boundary: evacuate pav, reciprocal Z, roundtrip ----
                    for b in range(b_sz):
                        nc.vector.tensor_copy(
                            av_u[b][:, ni * 512:(ni + 1) * 512], pav[b])
                    zrf = zc.tile([128, 512], F32, tag="zrf", name=f"zrf_{ni}")
                    nc.vector.reciprocal_approx_fast(zrf, zt)
                    zr = zc.tile([128, 512], BF16, tag="zr", name=f"zr_{ni}")
                    with nc.allow_low_precision(reason="1/Z bf16; ~4e-3 ok at 2e-2 gate"):
                        nc.vector.tensor_copy(zr, zrf)
                    for b in range(b_sz):
                        for h in range(HL):
                            s = b * HL + h
                            nc.gpsimd.dma_start(
                                out=zdram[b, h, 0, ni * 512:(ni + 1) * 512],
                                in_=zr[s * 32:s * 32 + 1, :])
                    for b in range(b_sz):
                        zb = zc.tile([128, 512], BF16, tag="zbb", name=f"zbb_{ni}_{b}")
                        for h in range(HL):
                            nc.gpsimd.dma_start(
                                out=zb[h * DH:(h + 1) * DH, :],
                                in_=zdram[b, h, :, ni * 512:(ni + 1) * 512]
                                .to_broadcast([DH, 512]))
                        with nc.allow_low_precision(reason="bf16 attention weights"):
                            nc.vector.tensor_mul(
                                av_n[b][:, ni * 512:(ni + 1) * 512],
                                av_u[b][:, ni * 512:(ni + 1) * 512],
                                zb)
            # ---- epilogue: last query-chunk's output projection ----
            with (
                tc.tile_pool(name="pout", bufs=4, space="PSUM") as pout,
                tc.tile_pool(name="ost2", bufs=6) as ost2,
            ):
                ni = nic - 1
                for c in range(2 * (512 // 128) * b_sz):
                    tb, half = c // 2, c % 2
                    b, rr = tb // (512 // 128), tb % (512 // 128)
                    r0 = ni * 512 + rr * 128
                    po = pout.tile([128, 512], F32, tag="po", name=f"poe_{c}")
                    nc.tensor.matmul(
                        po, av_n[b][:, r0:r0 + 128],
                        wo_full[:, half * 512:(half + 1) * 512],
                        start=True, stop=True,
                    )
                    os_ = ost2.tile([128, 512], BF16, tag="os", name=f"ose_{c}")
                    if c % 2 == 0:
                        nc.vector.tensor_copy(os_, po)
                    else:
                        nc.scalar.activation(out=os_, in_=po, func=AF.Copy)
                    nc.sync.dma_start(
                        out=out[b * n_sz + r0:b * n_sz + r0 + 128,
                                half * 512:(half + 1) * 512],
                        in_=os_)
    nc.compile()
    return nc


_NC_CACHE = {}


def _get_nc(key, **kw):
    if key not in _NC_CACHE:
        _NC_CACHE[key] = build(**kw)
    return _NC_CACHE[key]


def make_in_maps(x, rel_pos_bias, g, Wq, Wkv, Wo):
    b_sz, n_sz, dim = x.shape
    inner = Wq.shape[0]
    x = np.asarray(x, np.float32)
    xTh = np.ascontiguousarray(x.transpose(0, 2, 1)).astype(BF)  # [B, DIM, N]
    gv = np.asarray(g, np.float32).reshape(1, dim)
    Wq = np.asarray(Wq, np.float32) * gv
    Wkv = np.asarray(Wkv, np.float32) * gv
    scale = DH ** -0.5
    in_maps = []
    for c in range(NCORES):
        rs, re = c * IL, (c + 1) * IL
        wq_c = np.ascontiguousarray((Wq[rs:re, :] * scale).T).astype(BF)
        wk_c = np.ascontiguousarray(Wkv[rs:re, :].T).astype(BF)
        wv_c = np.ascontiguousarray(Wkv[inner + rs:inner + re, :].T).astype(BF)
        wo_c = np.ascontiguousarray(np.asarray(Wo)[:, rs:re].T).astype(BF)
        bias_c = np.exp(np.ascontiguousarray(
            np.asarray(rel_pos_bias)[0, c * HL:(c + 1) * HL].transpose(0, 2, 1)
        )).astype(BF)
        in_maps.append({
            "xT": xTh, "wqT": wq_c, "wkT": wk_c, "wvT": wv_c,
            "woT": wo_c, "biasT": bias_c,
        })
    return in_maps


def kernel(x, rel_pos_bias, g, Wq, Wkv, Wo):
    b_sz, n_sz, dim = x.shape
    nc = _get_nc((b_sz, n_sz, dim), b_sz=b_sz, n_sz=n_sz, dim=dim)
    in_maps = make_in_maps(x, rel_pos_bias, g, Wq, Wkv, Wo)
    res = run_bass_kernel_spmd(nc, in_maps, core_ids=list(range(NCORES)))
    acc = np.zeros((b_sz * n_sz, dim), np.float32)
    for r in res.results:
        acc += np.asarray(r["out"]).astype(np.float32)
    return np.ascontiguousarray(acc.reshape(b_sz, n_sz, dim))
